# revision 30
# baseline (speedup 1.0000x reference)
"""ACM-GCN single-layer kernel for Trainium2, 8 NeuronCores (SPMD).

Strategy (graph/data parallel):
- Nodes partitioned 12500/core (padded to 12544 = 98*128).
- Phase 1: h = x_shard @ [W_hp|W_lp|W_i] + b (bf16 PE matmul); h_hp/h_i kept
  in SBUF; deg_isqrt-prescaled bf16 table [12544, 128] written to DRAM.
- AllGather the table -> full [100352, 128] bf16 table per core.
- Phase 2: per dst tile, dma_gather the source rows of its edges (4 SWDGE
  queues, int16 indices via 4 source buckets), build one-hot selection
  matrices (tensor_scalar is_equal vs iota -> DVE 4x mode) and accumulate
  sel.T @ gathered in PSUM.  Self-edges appended so the self-loop term is
  folded into the aggregation.  Epilogue fuses HP/LP/I branches, gates and
  log_softmax.
"""
import numpy as np
import ml_dtypes

N_NODES = 100000
N_EDGES = 3200000
IN_DIM = 256
OUT_DIM = 64
NCORES = 8
P = 128
SHARD = N_NODES // NCORES            # 12500
NT = (SHARD + P - 1) // P            # 98 tiles
SHARD_PAD = NT * P                   # 12544
NTOT_PAD = SHARD_PAD * NCORES        # 100352
NBUCK = 4
BUCK = NTOT_PAD // NBUCK             # 25088 (< 32768, int16-safe)
D2 = 2 * OUT_DIM                     # 128 gathered feature dim (hp|lp)
DCAT = 3 * OUT_DIM                   # 192
SUPER = 2                            # dst tiles per gather group
BF16 = ml_dtypes.bfloat16


def _build_host_data(x, edge_index, W_hp, b_hp, W_lp, b_lp, W_i, b_i,
                     w_gh, b_gh, w_gl, b_gl, w_gi, b_gi):
    src = np.asarray(edge_index[0], dtype=np.int64)
    dst = np.asarray(edge_index[1], dtype=np.int64)

    deg = np.bincount(dst, minlength=N_NODES).astype(np.float64) + 1.0
    disqrt = (1.0 / np.sqrt(deg)).astype(np.float32)

    # self edges: with the prescaled table they contribute deg_inv * h
    allv = np.arange(N_NODES, dtype=np.int64)
    src = np.concatenate([src, allv])
    dst = np.concatenate([dst, allv])

    core = dst // SHARD
    dloc = dst - core * SHARD
    tile = dloc // P
    dstloc = (dloc % P).astype(np.int32)
    spad = (src // SHARD) * SHARD_PAD + (src % SHARD)
    buck = (spad // BUCK).astype(np.int32)
    idx16 = (spad - buck.astype(np.int64) * BUCK).astype(np.int16)

    key = ((core * NT + tile) * NBUCK + buck).astype(np.int64)
    order = np.argsort(key, kind="stable")
    idx16_s = idx16[order]
    dstloc_s = dstloc[order]
    cnts = np.bincount(key[order], minlength=NCORES * NT * NBUCK).reshape(
        NCORES, NT, NBUCK)

    NGRP = (NT + SUPER - 1) // SUPER
    # pack each (group, bucket)'s edges contiguously (tiles concatenated, one
    # pad-to-128 at the end); dl encodes tile-within-group*128 + dstloc
    grp_cnt = np.zeros((NCORES, NGRP, NBUCK), np.int64)
    for g in range(NGRP):
        ts = list(range(g * SUPER, min((g + 1) * SUPER, NT)))
        grp_cnt[:, g, :] = cnts[:, ts, :].sum(axis=1)
    C_gb = np.ceil(grp_cnt.max(axis=0) / P).astype(np.int64)       # [NGRP, NBUCK]
    NCHUNK_TOT = int(C_gb.sum())
    NIDX_TOT = NCHUNK_TOT * P
    grp_nidx = C_gb * P

    core_seg_start = np.cumsum(cnts.reshape(NCORES, -1), axis=1).reshape(
        NCORES, NT, NBUCK) - cnts
    core_base = np.concatenate([[0], np.cumsum(cnts.sum(axis=(1, 2)))])[:-1]

    idx_stream = np.zeros((NCORES, NIDX_TOT), np.int16)
    dl_stream = np.full((NCORES, NIDX_TOT), 300.0, np.float32)
    pos = 0
    for g in range(NGRP):
        ts = list(range(g * SUPER, min((g + 1) * SUPER, NT)))
        for b in range(NBUCK):
            n_pad = int(C_gb[g, b]) * P
            if n_pad == 0:
                continue
            for c in range(NCORES):
                p0 = pos
                for ti, t in enumerate(ts):
                    s0 = core_base[c] + core_seg_start[c, t, b]
                    n = int(cnts[c, t, b])
                    idx_stream[c, p0:p0 + n] = idx16_s[s0:s0 + n]
                    dl_stream[c, p0:p0 + n] = dstloc_s[s0:s0 + n] + ti * P
                    p0 += n
            pos += n_pad
    assert pos == NIDX_TOT

    idx_wrapped = np.zeros((NCORES, 128, NIDX_TOT // 16), np.int16)
    for c in range(NCORES):
        a = idx_stream[c].reshape(NIDX_TOT // 16, 16).T
        idx_wrapped[c] = np.tile(a, (8, 1))

    # matmul schedule: per (g, b, chunk) the set of group-local tiles it can
    # touch on any core; one streamed one-hot sel block per (chunk, tile) pair
    mm_sched = []          # (g, b, k, ti)
    grp_nmm = np.zeros(NGRP, np.int64)
    for g in range(NGRP):
        ts = list(range(g * SUPER, min((g + 1) * SUPER, NT)))
        nmm = 0
        for b in range(NBUCK):
            run_min = np.zeros(len(ts) + 1, np.int64)
            run_max = np.zeros(len(ts) + 1, np.int64)
            for ti, t in enumerate(ts):
                run_min[ti + 1] = run_min[ti] + cnts[:, t, b].min()
                run_max[ti + 1] = run_max[ti] + cnts[:, t, b].max()
            for k in range(int(C_gb[g, b])):
                ks, ke = k * P, (k + 1) * P
                for ti, t in enumerate(ts):
                    if ke > run_min[ti] and ks < run_max[ti + 1]:
                        mm_sched.append((g, b, k, ti))
                        nmm += 1
        grp_nmm[g] = nmm
    NMM_TOT = len(mm_sched)

    # chunk start offsets in the idx/dl stream, per (g, b)
    gb_off = {}
    off = 0
    for g in range(NGRP):
        for b in range(NBUCK):
            gb_off[(g, b)] = off
            off += int(C_gb[g, b]) * P

    ONE = np.float32(1.0).astype(BF16).view(np.uint16)
    sel_stream = np.zeros((NCORES, 128, NMM_TOT * P), np.uint16)
    dvals = np.arange(P, dtype=np.float32)
    for m, (g, b, k, ti) in enumerate(mm_sched):
        base = gb_off[(g, b)] + k * P
        for c in range(NCORES):
            col = dl_stream[c, base:base + P] - ti * P       # [128 edges]
            mask = (col >= 0) & (col < P)
            pp = np.nonzero(mask)[0]
            sel_stream[c, pp, m * P + col[pp].astype(np.int64)] = ONE
    sel_stream = sel_stream.view(BF16)

    W_cat = np.concatenate([W_hp, W_lp, W_i], axis=1).astype(np.float32)
    b_cat = np.concatenate([b_hp, b_lp, b_i]).astype(np.float32)
    wg_cat = np.concatenate([w_gh[:, 0], w_gl[:, 0], w_gi[:, 0]]).astype(np.float32)
    bg_cat = np.array([b_gh[0], b_gl[0], b_gi[0]], np.float32)

    xT = np.zeros((NCORES, IN_DIM, SHARD_PAD), BF16)
    disq_col = np.ones((NCORES, P, NT), np.float32)
    x = np.asarray(x, np.float32)
    for c in range(NCORES):
        xT[c, :, :SHARD] = x[c * SHARD:(c + 1) * SHARD].T.astype(BF16)
        d = np.ones(SHARD_PAD, np.float32)
        d[:SHARD] = disqrt[c * SHARD:(c + 1) * SHARD]
        disq_col[c] = d.reshape(NT, P).T

    consts = dict(
        Wcat=W_cat.astype(BF16),
        bias_rep=np.tile(b_cat[None, :], (P, 1)).astype(np.float32),
        wg_rep=np.tile(wg_cat[None, :], (P, 1)).astype(np.float32),
        bg_rep=np.tile(bg_cat[None, :], (P, 1)).astype(np.float32),
        iota=np.tile(np.arange(SUPER * P, dtype=np.float32)[None, :],
                     (P, 1)).astype(BF16),
    )
    meta = dict(C_gb=C_gb, grp_cnt=grp_cnt, cnts=cnts, grp_nidx=grp_nidx,
                NCHUNK_TOT=NCHUNK_TOT, NIDX_TOT=NIDX_TOT, NGRP=NGRP,
                mm_sched=mm_sched, grp_nmm=grp_nmm, NMM_TOT=NMM_TOT)
    per_core = dict(xT=xT, disq_col=disq_col, idx_wrapped=idx_wrapped,
                    sel_stream=sel_stream)
    return consts, meta, per_core


def _force_act_set():
    """Make every activation use the one table set that holds relu+exp+ln+copy
    (index preserved), so the kernel loads the ACT table exactly once."""
    import concourse.hw_specs as hw_specs
    if getattr(hw_specs, "_acm_patched", False):
        return
    orig = hw_specs.get_activation_tables

    def patched(module_arch):
        tabs = orig(module_arch)
        full = None
        for name, funcs in tabs.items():
            fn = {str(f) for f in funcs}
            if any("Exp" in f for f in fn) and any("Ln" in str(f) for f in fn) \
               and any("Relu" in f for f in fn):
                full = name
                break
        if full is None:
            return tabs
        keep = tabs[full]
        return {name: (funcs if name == full else (funcs & keep) - keep)
                for name, funcs in tabs.items()}

    hw_specs.get_activation_tables = patched
    import concourse.bacc as bacc_mod
    bacc_mod.get_activation_tables = patched
    hw_specs._acm_patched = True


def _build_bass(meta):
    import concourse.bacc as bacc
    import concourse.tile as tile
    from concourse import mybir

    _force_act_set()

    C_gb = meta["C_gb"]
    grp_cnt = meta["grp_cnt"]
    cnts = meta["cnts"]
    grp_nidx = meta["grp_nidx"]
    NCHUNK_TOT = meta["NCHUNK_TOT"]
    NIDX_TOT = meta["NIDX_TOT"]
    NGRP = meta["NGRP"]
    mm_sched = meta["mm_sched"]
    grp_nmm = meta["grp_nmm"]
    NMM_TOT = meta["NMM_TOT"]

    nc = bacc.Bacc("TRN2", target_bir_lowering=False, debug=False,
                   num_devices=NCORES, num_swdge_queues=4)

    f32, bf16, i16 = mybir.dt.float32, mybir.dt.bfloat16, mybir.dt.int16
    AF = mybir.ActivationFunctionType
    OP = mybir.AluOpType

    xT_in = nc.dram_tensor("xT", [IN_DIM, SHARD_PAD], bf16, kind="ExternalInput")
    disq_in = nc.dram_tensor("disq", [P, NT], f32, kind="ExternalInput")
    idx_in = nc.dram_tensor("idx", [P, NIDX_TOT // 16], i16, kind="ExternalInput")
    sel_in = nc.dram_tensor("sel", [P, NMM_TOT * P], bf16, kind="ExternalInput")
    Wcat_in = nc.dram_tensor("Wcat", [IN_DIM, DCAT], bf16, kind="ExternalInput")
    bias_in = nc.dram_tensor("bias_rep", [P, DCAT], f32, kind="ExternalInput")
    wg_in = nc.dram_tensor("wg_rep", [P, DCAT], f32, kind="ExternalInput")
    bg_in = nc.dram_tensor("bg_rep", [P, 3], f32, kind="ExternalInput")
    out_ext = nc.dram_tensor("out", [SHARD_PAD, OUT_DIM], f32, kind="ExternalOutput")

    table_own = nc.dram_tensor("table_own", [SHARD_PAD, D2], bf16)
    wu_src = nc.dram_tensor("wu_src", [128, D2], bf16)
    wu_cin = nc.dram_tensor("wu_cin", [128, 8], bf16)
    wu_cout = nc.dram_tensor("wu_cout", [NCORES * 128, 8], bf16, addr_space="Shared")
    table_full = nc.dram_tensor("table_full", [NTOT_PAD, D2], bf16, addr_space="Shared")

    with tile.TileContext(nc) as tc:
        with (
            tc.tile_pool(name="consts", bufs=1) as consts,
            tc.tile_pool(name="hpool", bufs=1) as hpool,
        ):
            # --- warmups: first collective and first SWDGE gather are cold
            # (ncfw staging / Q7 library load); run tiny dummies early so the
            # real ones hit warm paths, overlapped with phase 1 ---
            wu_idx = consts.tile([P, 8], mybir.dt.int16)
            nc.gpsimd.memset(wu_idx[:], 0)
            wu_out = consts.tile([P, 1, D2], bf16)
            nc.gpsimd.dma_gather(
                out_ap=wu_out[:], in_ap=wu_src[:], idxs_ap=wu_idx[:],
                num_idxs=128, num_idxs_reg=128, elem_size=D2,
                single_packet=False, queue_num=0)
            nc.gpsimd.collective_compute(
                "AllGather", mybir.AluOpType.bypass,
                replica_groups=[list(range(NCORES))],
                ins=[wu_cin[:]], outs=[wu_cout[:]],
            )
            W0 = consts.tile([P, DCAT], bf16)
            W1 = consts.tile([P, DCAT], bf16)
            nc.sync.dma_start(out=W0[:], in_=Wcat_in[0:P, :])
            nc.sync.dma_start(out=W1[:], in_=Wcat_in[P:2 * P, :])
            bias_t = consts.tile([P, DCAT], f32)
            nc.sync.dma_start(out=bias_t[:], in_=bias_in[:])
            wg_t = consts.tile([P, DCAT], f32)
            nc.sync.dma_start(out=wg_t[:], in_=wg_in[:])
            bg_t = consts.tile([P, 3], f32)
            nc.sync.dma_start(out=bg_t[:], in_=bg_in[:])
            disq_t = consts.tile([P, NT], f32)
            nc.sync.dma_start(out=disq_t[:], in_=disq_in[:])
            ndisq_t = consts.tile([P, NT], f32)
            nc.vector.tensor_scalar_mul(out=ndisq_t[:], in0=disq_t[:], scalar1=-1.0)

            # h_slab holds [h_hp | h_i] per tile: [128, 98*128] bf16
            h_slab = hpool.tile([P, NT * D2], bf16)

            # ---- phase 1 ----
            with (
                tc.tile_pool(name="xt", bufs=1) as xtp,
                tc.tile_pool(name="p1", bufs=3) as p1,
                tc.tile_pool(name="p1ps", bufs=2, space="PSUM") as p1ps,
            ):
                xt0 = xtp.tile([P, SHARD_PAD], bf16)
                xt1 = xtp.tile([P, SHARD_PAD], bf16)
                nc.sync.dma_start(out=xt0[:], in_=xT_in[0:P, :])
                nc.sync.dma_start(out=xt1[:], in_=xT_in[P:2 * P, :])
                for t in range(NT):
                    ps = p1ps.tile([P, DCAT], f32, tag="p1ps")
                    nc.tensor.matmul(out=ps[:], lhsT=xt0[:, t * P:(t + 1) * P],
                                     rhs=W0[:], start=True, stop=False)
                    nc.tensor.matmul(out=ps[:], lhsT=xt1[:, t * P:(t + 1) * P],
                                     rhs=W1[:], start=False, stop=True)
                    sc = p1.tile([P, DCAT], f32, tag="sc")
                    nc.vector.tensor_add(out=sc[:], in0=ps[:], in1=bias_t[:])
                    # h_slab tile t = [hp (0:64) | i (128:192)]
                    sc3 = sc[:].rearrange("p (a b) -> p a b", a=3)
                    nc.vector.tensor_copy(
                        out=h_slab[:, t * D2:(t + 1) * D2].rearrange(
                            "p (a b) -> p a b", a=2),
                        in_=sc3[:, 0::2, :])
                    tab = p1.tile([P, D2], bf16, tag="tab")
                    nc.scalar.activation(out=tab[:], in_=sc[:, 0:D2],
                                         func=AF.Copy, scale=disq_t[:, t:t + 1])
                    nc.sync.dma_start(out=table_own[t * P:(t + 1) * P, :], in_=tab[:])

            nc.gpsimd.collective_compute(
                "AllGather", mybir.AluOpType.bypass,
                replica_groups=[list(range(NCORES))],
                ins=[table_own[:]], outs=[table_full[:]],
            )

            # ---- phase 2 ----
            with (
                tc.tile_pool(name="gath", bufs=4) as gpool,
                tc.tile_pool(name="idxp", bufs=6) as idxp,
                tc.tile_pool(name="sel", bufs=3) as selp,
                tc.tile_pool(name="ep", bufs=3) as ep,
                tc.tile_pool(name="ps2", bufs=8, space="PSUM") as ps2,
            ):
                idx_off = 0
                mm_off = 0
                qn = 0
                max_nmm = int(grp_nmm.max())
                for g in range(NGRP):
                    ts = list(range(g * SUPER, min((g + 1) * SUPER, NT)))
                    nidx_g = int(grp_nidx[g].sum())
                    if nidx_g == 0:
                        continue
                    icols = nidx_g // 16
                    idx_t = idxp.tile([P, icols], i16, tag="idx")
                    nc.sync.dma_start(
                        out=idx_t[:],
                        in_=idx_in[:, idx_off // 16:idx_off // 16 + icols])
                    gts = []
                    off_in_g = 0
                    for b in range(NBUCK):
                        nb = int(grp_nidx[g, b])
                        if nb == 0:
                            gts.append(None)
                            continue
                        gt = gpool.tile([P, nb // P, D2], bf16, tag=f"g{b}")
                        nc.gpsimd.dma_gather(
                            out_ap=gt[:],
                            in_ap=table_full[b * BUCK:(b + 1) * BUCK, :],
                            idxs_ap=idx_t[:, off_in_g // 16:(off_in_g + nb) // 16],
                            num_idxs=nb, num_idxs_reg=nb, elem_size=D2,
                            single_packet=False, queue_num=qn,
                        )
                        qn = (qn + 1) % 4
                        gts.append(gt)
                        off_in_g += nb
                    idx_off += nidx_g

                    nmm_g = int(grp_nmm[g])
                    selst = selp.tile([P, max_nmm, P], bf16, tag="selst")
                    sched_g = mm_sched[mm_off:mm_off + nmm_g]
                    nc.sync.dma_start(
                        out=selst[:, 0:nmm_g, :],
                        in_=sel_in[:, mm_off * P:(mm_off + nmm_g) * P])
                    n_touch = {t: 0 for t in ts}
                    for (gg, b, k, ti) in sched_g:
                        n_touch[ts[ti]] += 1
                    psums = {}
                    first = {t: True for t in ts}
                    done = {t: 0 for t in ts}
                    for m, (gg, b, k, ti) in enumerate(sched_g):
                        t = ts[ti]
                        if t not in psums:
                            psums[t] = ps2.tile([P, D2], f32, tag="acc",
                                                name=f"acc{t}")
                        done[t] += 1
                        nc.tensor.matmul(
                            out=psums[t][:], lhsT=selst[:, m, :],
                            rhs=gts[b][:, k, :],
                            start=first[t],
                            stop=(done[t] == n_touch[t]))
                        first[t] = False
                    mm_off += nmm_g

                    for t in ts:
                        acc = psums[t]
                        hof = t * D2
                        Hcat = ep.tile([P, DCAT], f32, tag="Hcat")
                        nc.vector.scalar_tensor_tensor(
                            out=Hcat[:, 0:OUT_DIM], in0=acc[:, 0:OUT_DIM],
                            scalar=ndisq_t[:, t:t + 1],
                            in1=h_slab[:, hof:hof + OUT_DIM],
                            op0=OP.mult, op1=OP.add)
                        nc.scalar.activation(out=Hcat[:, 0:OUT_DIM],
                                             in_=Hcat[:, 0:OUT_DIM], func=AF.Relu)
                        nc.scalar.activation(out=Hcat[:, OUT_DIM:D2],
                                             in_=acc[:, OUT_DIM:D2], func=AF.Relu,
                                             scale=disq_t[:, t:t + 1])
                        nc.scalar.activation(out=Hcat[:, D2:DCAT],
                                             in_=h_slab[:, hof + OUT_DIM:hof + D2],
                                             func=AF.Relu)
                        gm = ep.tile([P, DCAT], f32, tag="gm")
                        nc.vector.tensor_tensor(out=gm[:], in0=Hcat[:], in1=wg_t[:],
                                                op=OP.mult)
                        g3 = ep.tile([P, 4], f32, tag="g3")
                        nc.vector.reduce_sum(
                            out=g3[:, 0:3],
                            in_=gm[:].rearrange("p (a b) -> p a b", a=3),
                            axis=mybir.AxisListType.X)
                        nc.vector.tensor_add(out=g3[:, 0:3], in0=g3[:, 0:3],
                                             in1=bg_t[:])
                        o = ep.tile([P, OUT_DIM], f32, tag="o")
                        nc.scalar.activation(out=o[:], in_=Hcat[:, 0:OUT_DIM],
                                             func=AF.Copy, scale=g3[:, 0:1])
                        nc.vector.scalar_tensor_tensor(
                            out=o[:], in0=Hcat[:, OUT_DIM:D2], scalar=g3[:, 1:2],
                            in1=o[:], op0=OP.mult, op1=OP.add)
                        nc.vector.scalar_tensor_tensor(
                            out=o[:], in0=Hcat[:, D2:DCAT], scalar=g3[:, 2:3],
                            in1=o[:], op0=OP.mult, op1=OP.add)
                        mx = ep.tile([P, 4], f32, tag="mx")
                        nc.vector.tensor_reduce(out=mx[:, 0:1], in_=o[:],
                                                op=OP.max,
                                                axis=mybir.AxisListType.X,
                                                negate=True)
                        et = ep.tile([P, OUT_DIM], f32, tag="et")
                        nc.scalar.activation(out=et[:], in_=o[:], func=AF.Exp,
                                             bias=mx[:, 0:1], scale=1.0,
                                             accum_out=mx[:, 1:2])
                        nc.scalar.activation(out=mx[:, 2:3], in_=mx[:, 1:2],
                                             func=AF.Ln)
                        fin = ep.tile([P, OUT_DIM], f32, tag="fin")
                        nc.vector.tensor_scalar(
                            out=fin[:], in0=o[:], scalar1=mx[:, 0:1],
                            scalar2=mx[:, 2:3], op0=OP.add, op1=OP.subtract)
                        nc.sync.dma_start(out=out_ext[t * P:(t + 1) * P, :],
                                          in_=fin[:])

    nc.compile()
    return nc


def kernel(**inputs) -> np.ndarray:
    consts, meta, per_core = _build_host_data(**inputs)
    nc = _build_bass(meta)

    in_maps = []
    for c in range(NCORES):
        in_maps.append(dict(
            xT=np.ascontiguousarray(per_core["xT"][c]),
            disq=np.ascontiguousarray(per_core["disq_col"][c]),
            idx=np.ascontiguousarray(per_core["idx_wrapped"][c]),
            sel=per_core["sel_stream"][c].reshape(P, -1),
            Wcat=consts["Wcat"], bias_rep=consts["bias_rep"],
            wg_rep=consts["wg_rep"], bg_rep=consts["bg_rep"],
        ))

    from concourse.bass_utils import run_bass_kernel_spmd
    res = run_bass_kernel_spmd(nc, in_maps, core_ids=list(range(NCORES)))
    out = np.concatenate([res.results[c]["out"][:SHARD] for c in range(NCORES)],
                         axis=0)
    return out.astype(np.float32)


if __name__ == "__main__":
    import reference
    ins = {k: np.asarray(v) for k, v in reference.setup_inputs().items()}
    got = kernel(**ins)
    exp = np.asarray(reference.reference(**reference.setup_inputs()))
    rel = np.linalg.norm(got - exp) / np.linalg.norm(exp)
    print("Relative error:", rel)


# revision 31
# speedup vs baseline: 1.0517x; 1.0517x over previous
"""ACM-GCN single-layer kernel for Trainium2, 8 NeuronCores (SPMD).

Strategy (graph/data parallel):
- Nodes partitioned 12500/core (padded to 12544 = 98*128).
- Phase 1: h = x_shard @ [W_hp|W_lp|W_i] + b (bf16 PE matmul); h_hp/h_i kept
  in SBUF; deg_isqrt-prescaled bf16 table [12544, 128] written to DRAM.
- AllGather the table -> full [100352, 128] bf16 table per core.
- Phase 2: per dst tile, dma_gather the source rows of its edges (4 SWDGE
  queues, int16 indices via 4 source buckets), build one-hot selection
  matrices (tensor_scalar is_equal vs iota -> DVE 4x mode) and accumulate
  sel.T @ gathered in PSUM.  Self-edges appended so the self-loop term is
  folded into the aggregation.  Epilogue fuses HP/LP/I branches, gates and
  log_softmax.
"""
import numpy as np
import ml_dtypes

N_NODES = 100000
N_EDGES = 3200000
IN_DIM = 256
OUT_DIM = 64
NCORES = 8
P = 128
SHARD = N_NODES // NCORES            # 12500
NT = (SHARD + P - 1) // P            # 98 tiles
SHARD_PAD = NT * P                   # 12544
NTOT_PAD = SHARD_PAD * NCORES        # 100352
NBUCK = 4
BUCK = NTOT_PAD // NBUCK             # 25088 (< 32768, int16-safe)
D2 = 2 * OUT_DIM                     # 128 gathered feature dim (hp|lp)
DCAT = 3 * OUT_DIM                   # 192
SUPER = 2                            # dst tiles per gather group
HNT = NT // 2                        # 49 tiles per half
HALF = HNT * P                       # 6272 rows per half
BF16 = ml_dtypes.bfloat16


def _build_host_data(x, edge_index, W_hp, b_hp, W_lp, b_lp, W_i, b_i,
                     w_gh, b_gh, w_gl, b_gl, w_gi, b_gi):
    src = np.asarray(edge_index[0], dtype=np.int64)
    dst = np.asarray(edge_index[1], dtype=np.int64)

    deg = np.bincount(dst, minlength=N_NODES).astype(np.float64) + 1.0
    disqrt = (1.0 / np.sqrt(deg)).astype(np.float32)

    # self edges: with the prescaled table they contribute deg_inv * h
    allv = np.arange(N_NODES, dtype=np.int64)
    src = np.concatenate([src, allv])
    dst = np.concatenate([dst, allv])

    core = dst // SHARD
    dloc = dst - core * SHARD
    tile = dloc // P
    dstloc = (dloc % P).astype(np.int32)
    # half-major padded global index: the table is all-gathered as two
    # per-half collectives, so global row = half*8*HALF + core*HALF + loc%HALF
    s_core = src // SHARD
    s_loc = src % SHARD
    s_half = s_loc // HALF
    spad = s_half * (NCORES * HALF) + s_core * HALF + (s_loc - s_half * HALF)
    buck = (spad // BUCK).astype(np.int32)
    idx16 = (spad - buck.astype(np.int64) * BUCK).astype(np.int16)

    key = ((core * NT + tile) * NBUCK + buck).astype(np.int64)
    order = np.argsort(key, kind="stable")
    idx16_s = idx16[order]
    dstloc_s = dstloc[order]
    cnts = np.bincount(key[order], minlength=NCORES * NT * NBUCK).reshape(
        NCORES, NT, NBUCK)

    NGRP = (NT + SUPER - 1) // SUPER
    # pack each (group, bucket)'s edges contiguously (tiles concatenated, one
    # pad-to-128 at the end); dl encodes tile-within-group*128 + dstloc
    grp_cnt = np.zeros((NCORES, NGRP, NBUCK), np.int64)
    for g in range(NGRP):
        ts = list(range(g * SUPER, min((g + 1) * SUPER, NT)))
        grp_cnt[:, g, :] = cnts[:, ts, :].sum(axis=1)
    C_gb = np.ceil(grp_cnt.max(axis=0) / P).astype(np.int64)       # [NGRP, NBUCK]
    NCHUNK_TOT = int(C_gb.sum())
    NIDX_TOT = NCHUNK_TOT * P
    grp_nidx = C_gb * P

    core_seg_start = np.cumsum(cnts.reshape(NCORES, -1), axis=1).reshape(
        NCORES, NT, NBUCK) - cnts
    core_base = np.concatenate([[0], np.cumsum(cnts.sum(axis=(1, 2)))])[:-1]

    idx_stream = np.zeros((NCORES, NIDX_TOT), np.int16)
    dl_stream = np.full((NCORES, NIDX_TOT), 300.0, np.float32)
    pos = 0
    for g in range(NGRP):
        ts = list(range(g * SUPER, min((g + 1) * SUPER, NT)))
        for b in range(NBUCK):
            n_pad = int(C_gb[g, b]) * P
            if n_pad == 0:
                continue
            for c in range(NCORES):
                p0 = pos
                for ti, t in enumerate(ts):
                    s0 = core_base[c] + core_seg_start[c, t, b]
                    n = int(cnts[c, t, b])
                    idx_stream[c, p0:p0 + n] = idx16_s[s0:s0 + n]
                    dl_stream[c, p0:p0 + n] = dstloc_s[s0:s0 + n] + ti * P
                    p0 += n
            pos += n_pad
    assert pos == NIDX_TOT

    idx_wrapped = np.zeros((NCORES, 128, NIDX_TOT // 16), np.int16)
    for c in range(NCORES):
        a = idx_stream[c].reshape(NIDX_TOT // 16, 16).T
        idx_wrapped[c] = np.tile(a, (8, 1))

    # matmul schedule: per (g, b, chunk) the set of group-local tiles it can
    # touch on any core; one streamed one-hot sel block per (chunk, tile) pair
    mm_sched = []          # (g, b, k, ti)
    grp_nmm = np.zeros(NGRP, np.int64)
    for g in range(NGRP):
        ts = list(range(g * SUPER, min((g + 1) * SUPER, NT)))
        nmm = 0
        for b in range(NBUCK):
            run_min = np.zeros(len(ts) + 1, np.int64)
            run_max = np.zeros(len(ts) + 1, np.int64)
            for ti, t in enumerate(ts):
                run_min[ti + 1] = run_min[ti] + cnts[:, t, b].min()
                run_max[ti + 1] = run_max[ti] + cnts[:, t, b].max()
            for k in range(int(C_gb[g, b])):
                ks, ke = k * P, (k + 1) * P
                for ti, t in enumerate(ts):
                    if ke > run_min[ti] and ks < run_max[ti + 1]:
                        mm_sched.append((g, b, k, ti))
                        nmm += 1
        grp_nmm[g] = nmm
    NMM_TOT = len(mm_sched)

    # chunk start offsets in the idx/dl stream, per (g, b)
    gb_off = {}
    off = 0
    for g in range(NGRP):
        for b in range(NBUCK):
            gb_off[(g, b)] = off
            off += int(C_gb[g, b]) * P

    ONE = np.float32(1.0).astype(BF16).view(np.uint16)
    sel_stream = np.zeros((NCORES, 128, NMM_TOT * P), np.uint16)
    dvals = np.arange(P, dtype=np.float32)
    for m, (g, b, k, ti) in enumerate(mm_sched):
        base = gb_off[(g, b)] + k * P
        for c in range(NCORES):
            col = dl_stream[c, base:base + P] - ti * P       # [128 edges]
            mask = (col >= 0) & (col < P)
            pp = np.nonzero(mask)[0]
            sel_stream[c, pp, m * P + col[pp].astype(np.int64)] = ONE
    sel_stream = sel_stream.view(BF16)

    W_cat = np.concatenate([W_hp, W_lp, W_i], axis=1).astype(np.float32)
    b_cat = np.concatenate([b_hp, b_lp, b_i]).astype(np.float32)
    wg_cat = np.concatenate([w_gh[:, 0], w_gl[:, 0], w_gi[:, 0]]).astype(np.float32)
    bg_cat = np.array([b_gh[0], b_gl[0], b_gi[0]], np.float32)

    xT = np.zeros((NCORES, IN_DIM, SHARD_PAD), BF16)
    disq_col = np.ones((NCORES, P, NT), np.float32)
    x = np.asarray(x, np.float32)
    for c in range(NCORES):
        xT[c, :, :SHARD] = x[c * SHARD:(c + 1) * SHARD].T.astype(BF16)
        d = np.ones(SHARD_PAD, np.float32)
        d[:SHARD] = disqrt[c * SHARD:(c + 1) * SHARD]
        disq_col[c] = d.reshape(NT, P).T

    consts = dict(
        Wcat=W_cat.astype(BF16),
        bias_rep=np.tile(b_cat[None, :], (P, 1)).astype(np.float32),
        wg_rep=np.tile(wg_cat[None, :], (P, 1)).astype(np.float32),
        bg_rep=np.tile(bg_cat[None, :], (P, 1)).astype(np.float32),
        iota=np.tile(np.arange(SUPER * P, dtype=np.float32)[None, :],
                     (P, 1)).astype(BF16),
    )
    meta = dict(C_gb=C_gb, grp_cnt=grp_cnt, cnts=cnts, grp_nidx=grp_nidx,
                NCHUNK_TOT=NCHUNK_TOT, NIDX_TOT=NIDX_TOT, NGRP=NGRP,
                mm_sched=mm_sched, grp_nmm=grp_nmm, NMM_TOT=NMM_TOT)
    per_core = dict(xT=xT, disq_col=disq_col, idx_wrapped=idx_wrapped,
                    sel_stream=sel_stream)
    return consts, meta, per_core


def _force_act_set():
    """Make every activation use the one table set that holds relu+exp+ln+copy
    (index preserved), so the kernel loads the ACT table exactly once."""
    import concourse.hw_specs as hw_specs
    if getattr(hw_specs, "_acm_patched", False):
        return
    orig = hw_specs.get_activation_tables

    def patched(module_arch):
        tabs = orig(module_arch)
        full = None
        for name, funcs in tabs.items():
            fn = {str(f) for f in funcs}
            if any("Exp" in f for f in fn) and any("Ln" in str(f) for f in fn) \
               and any("Relu" in f for f in fn):
                full = name
                break
        if full is None:
            return tabs
        keep = tabs[full]
        return {name: (funcs if name == full else (funcs & keep) - keep)
                for name, funcs in tabs.items()}

    hw_specs.get_activation_tables = patched
    import concourse.bacc as bacc_mod
    bacc_mod.get_activation_tables = patched
    hw_specs._acm_patched = True


def _build_bass(meta):
    import concourse.bacc as bacc
    import concourse.tile as tile
    from concourse import mybir

    _force_act_set()

    C_gb = meta["C_gb"]
    grp_cnt = meta["grp_cnt"]
    cnts = meta["cnts"]
    grp_nidx = meta["grp_nidx"]
    NCHUNK_TOT = meta["NCHUNK_TOT"]
    NIDX_TOT = meta["NIDX_TOT"]
    NGRP = meta["NGRP"]
    mm_sched = meta["mm_sched"]
    grp_nmm = meta["grp_nmm"]
    NMM_TOT = meta["NMM_TOT"]

    nc = bacc.Bacc("TRN2", target_bir_lowering=False, debug=False,
                   num_devices=NCORES, num_swdge_queues=4)

    f32, bf16, i16 = mybir.dt.float32, mybir.dt.bfloat16, mybir.dt.int16
    AF = mybir.ActivationFunctionType
    OP = mybir.AluOpType

    xT_in = nc.dram_tensor("xT", [IN_DIM, SHARD_PAD], bf16, kind="ExternalInput")
    disq_in = nc.dram_tensor("disq", [P, NT], f32, kind="ExternalInput")
    idx_in = nc.dram_tensor("idx", [P, NIDX_TOT // 16], i16, kind="ExternalInput")
    sel_in = nc.dram_tensor("sel", [P, NMM_TOT * P], bf16, kind="ExternalInput")
    Wcat_in = nc.dram_tensor("Wcat", [IN_DIM, DCAT], bf16, kind="ExternalInput")
    bias_in = nc.dram_tensor("bias_rep", [P, DCAT], f32, kind="ExternalInput")
    wg_in = nc.dram_tensor("wg_rep", [P, DCAT], f32, kind="ExternalInput")
    bg_in = nc.dram_tensor("bg_rep", [P, 3], f32, kind="ExternalInput")
    out_ext = nc.dram_tensor("out", [SHARD_PAD, OUT_DIM], f32, kind="ExternalOutput")

    table_own_a = nc.dram_tensor("table_own_a", [HALF, D2], bf16)
    table_own_b = nc.dram_tensor("table_own_b", [SHARD_PAD - HALF, D2], bf16)
    wu_src = nc.dram_tensor("wu_src", [128, D2], bf16)
    wu_cin = nc.dram_tensor("wu_cin", [128, 8], bf16)
    wu_cout = nc.dram_tensor("wu_cout", [NCORES * 128, 8], bf16, addr_space="Shared")
    table_full_a = nc.dram_tensor("table_full_a", [NCORES * HALF, D2], bf16,
                                  addr_space="Shared")
    table_full_b = nc.dram_tensor("table_full_b", [NTOT_PAD - NCORES * HALF, D2],
                                  bf16, addr_space="Shared")

    with tile.TileContext(nc) as tc:
        with (
            tc.tile_pool(name="consts", bufs=1) as consts,
            tc.tile_pool(name="hpool", bufs=1) as hpool,
        ):
            # --- warmups: first collective and first SWDGE gather are cold
            # (ncfw staging / Q7 library load); run tiny dummies early so the
            # real ones hit warm paths, overlapped with phase 1 ---
            wu_idx = consts.tile([P, 8], mybir.dt.int16)
            nc.gpsimd.memset(wu_idx[:], 0)
            wu_out = consts.tile([P, 1, D2], bf16)
            nc.gpsimd.dma_gather(
                out_ap=wu_out[:], in_ap=wu_src[:], idxs_ap=wu_idx[:],
                num_idxs=128, num_idxs_reg=128, elem_size=D2,
                single_packet=False, queue_num=0)
            nc.gpsimd.collective_compute(
                "AllGather", mybir.AluOpType.bypass,
                replica_groups=[list(range(NCORES))],
                ins=[wu_cin[:]], outs=[wu_cout[:]],
            )
            W0 = consts.tile([P, DCAT], bf16)
            W1 = consts.tile([P, DCAT], bf16)
            nc.sync.dma_start(out=W0[:], in_=Wcat_in[0:P, :])
            nc.sync.dma_start(out=W1[:], in_=Wcat_in[P:2 * P, :])
            bias_t = consts.tile([P, DCAT], f32)
            nc.sync.dma_start(out=bias_t[:], in_=bias_in[:])
            wg_t = consts.tile([P, DCAT], f32)
            nc.sync.dma_start(out=wg_t[:], in_=wg_in[:])
            bg_t = consts.tile([P, 3], f32)
            nc.sync.dma_start(out=bg_t[:], in_=bg_in[:])
            disq_t = consts.tile([P, NT], f32)
            nc.sync.dma_start(out=disq_t[:], in_=disq_in[:])
            ndisq_t = consts.tile([P, NT], f32)
            nc.vector.tensor_scalar_mul(out=ndisq_t[:], in0=disq_t[:], scalar1=-1.0)

            # h_slab holds [h_hp | h_i] per tile: [128, 98*128] bf16
            h_slab = hpool.tile([P, NT * D2], bf16)

            # ---- phase 1 ----
            with (
                tc.tile_pool(name="xt", bufs=1) as xtp,
                tc.tile_pool(name="p1", bufs=3) as p1,
                tc.tile_pool(name="p1ps", bufs=2, space="PSUM") as p1ps,
            ):
                xt0 = xtp.tile([P, SHARD_PAD], bf16)
                xt1 = xtp.tile([P, SHARD_PAD], bf16)
                nc.sync.dma_start(out=xt0[:], in_=xT_in[0:P, :])
                nc.sync.dma_start(out=xt1[:], in_=xT_in[P:2 * P, :])
                for t in range(NT):
                    ps = p1ps.tile([P, DCAT], f32, tag="p1ps")
                    nc.tensor.matmul(out=ps[:], lhsT=xt0[:, t * P:(t + 1) * P],
                                     rhs=W0[:], start=True, stop=False)
                    nc.tensor.matmul(out=ps[:], lhsT=xt1[:, t * P:(t + 1) * P],
                                     rhs=W1[:], start=False, stop=True)
                    sc = p1.tile([P, DCAT], f32, tag="sc")
                    nc.vector.tensor_add(out=sc[:], in0=ps[:], in1=bias_t[:])
                    # h_slab tile t = [hp (0:64) | i (128:192)]
                    sc3 = sc[:].rearrange("p (a b) -> p a b", a=3)
                    nc.vector.tensor_copy(
                        out=h_slab[:, t * D2:(t + 1) * D2].rearrange(
                            "p (a b) -> p a b", a=2),
                        in_=sc3[:, 0::2, :])
                    tab = p1.tile([P, D2], bf16, tag="tab")
                    nc.scalar.activation(out=tab[:], in_=sc[:, 0:D2],
                                         func=AF.Copy, scale=disq_t[:, t:t + 1])
                    if t < HNT:
                        nc.sync.dma_start(out=table_own_a[t * P:(t + 1) * P, :],
                                          in_=tab[:])
                    else:
                        tb = t - HNT
                        nc.sync.dma_start(out=table_own_b[tb * P:(tb + 1) * P, :],
                                          in_=tab[:])
                    if t == HNT - 1:
                        nc.gpsimd.collective_compute(
                            "AllGather", mybir.AluOpType.bypass,
                            replica_groups=[list(range(NCORES))],
                            ins=[table_own_a[:]], outs=[table_full_a[:]],
                        )

            nc.gpsimd.collective_compute(
                "AllGather", mybir.AluOpType.bypass,
                replica_groups=[list(range(NCORES))],
                ins=[table_own_b[:]], outs=[table_full_b[:]],
            )

            # ---- phase 2 ----
            with (
                tc.tile_pool(name="gath", bufs=4) as gpool,
                tc.tile_pool(name="idxp", bufs=6) as idxp,
                tc.tile_pool(name="sel", bufs=2) as selp,
                tc.tile_pool(name="ep", bufs=3) as ep,
                tc.tile_pool(name="ps2", bufs=8, space="PSUM") as ps2,
            ):
                idx_off = 0
                mm_off = 0
                qn = 0
                max_nmm = int(grp_nmm.max())
                for g in range(NGRP):
                    ts = list(range(g * SUPER, min((g + 1) * SUPER, NT)))
                    nidx_g = int(grp_nidx[g].sum())
                    if nidx_g == 0:
                        continue
                    icols = nidx_g // 16
                    idx_t = idxp.tile([P, icols], i16, tag="idx")
                    nc.sync.dma_start(
                        out=idx_t[:],
                        in_=idx_in[:, idx_off // 16:idx_off // 16 + icols])
                    gts = []
                    off_in_g = 0
                    for b in range(NBUCK):
                        nb = int(grp_nidx[g, b])
                        if nb == 0:
                            gts.append(None)
                            continue
                        gt = gpool.tile([P, nb // P, D2], bf16, tag=f"g{b}")
                        if b < 2:
                            src_tab = table_full_a[b * BUCK:(b + 1) * BUCK, :]
                        else:
                            src_tab = table_full_b[(b - 2) * BUCK:(b - 1) * BUCK, :]
                        nc.gpsimd.dma_gather(
                            out_ap=gt[:],
                            in_ap=src_tab,
                            idxs_ap=idx_t[:, off_in_g // 16:(off_in_g + nb) // 16],
                            num_idxs=nb, num_idxs_reg=nb, elem_size=D2,
                            single_packet=False, queue_num=qn,
                        )
                        qn = (qn + 1) % 4
                        gts.append(gt)
                        off_in_g += nb
                    idx_off += nidx_g

                    nmm_g = int(grp_nmm[g])
                    selst = selp.tile([P, max_nmm, P], bf16, tag="selst")
                    sched_g = mm_sched[mm_off:mm_off + nmm_g]
                    nc.sync.dma_start(
                        out=selst[:, 0:nmm_g, :],
                        in_=sel_in[:, mm_off * P:(mm_off + nmm_g) * P])
                    n_touch = {t: 0 for t in ts}
                    for (gg, b, k, ti) in sched_g:
                        n_touch[ts[ti]] += 1
                    psums = {}
                    first = {t: True for t in ts}
                    done = {t: 0 for t in ts}
                    for m, (gg, b, k, ti) in enumerate(sched_g):
                        t = ts[ti]
                        if t not in psums:
                            psums[t] = ps2.tile([P, D2], f32, tag="acc",
                                                name=f"acc{t}")
                        done[t] += 1
                        nc.tensor.matmul(
                            out=psums[t][:], lhsT=selst[:, m, :],
                            rhs=gts[b][:, k, :],
                            start=first[t],
                            stop=(done[t] == n_touch[t]))
                        first[t] = False
                    mm_off += nmm_g

                    for t in ts:
                        acc = psums[t]
                        hof = t * D2
                        Hcat = ep.tile([P, DCAT], f32, tag="Hcat")
                        nc.vector.scalar_tensor_tensor(
                            out=Hcat[:, 0:OUT_DIM], in0=acc[:, 0:OUT_DIM],
                            scalar=ndisq_t[:, t:t + 1],
                            in1=h_slab[:, hof:hof + OUT_DIM],
                            op0=OP.mult, op1=OP.add)
                        nc.scalar.activation(out=Hcat[:, 0:OUT_DIM],
                                             in_=Hcat[:, 0:OUT_DIM], func=AF.Relu)
                        nc.scalar.activation(out=Hcat[:, OUT_DIM:D2],
                                             in_=acc[:, OUT_DIM:D2], func=AF.Relu,
                                             scale=disq_t[:, t:t + 1])
                        nc.scalar.activation(out=Hcat[:, D2:DCAT],
                                             in_=h_slab[:, hof + OUT_DIM:hof + D2],
                                             func=AF.Relu)
                        gm = ep.tile([P, DCAT], f32, tag="gm")
                        nc.vector.tensor_tensor(out=gm[:], in0=Hcat[:], in1=wg_t[:],
                                                op=OP.mult)
                        g3 = ep.tile([P, 4], f32, tag="g3")
                        nc.vector.reduce_sum(
                            out=g3[:, 0:3],
                            in_=gm[:].rearrange("p (a b) -> p a b", a=3),
                            axis=mybir.AxisListType.X)
                        nc.vector.tensor_add(out=g3[:, 0:3], in0=g3[:, 0:3],
                                             in1=bg_t[:])
                        o = ep.tile([P, OUT_DIM], f32, tag="o")
                        nc.scalar.activation(out=o[:], in_=Hcat[:, 0:OUT_DIM],
                                             func=AF.Copy, scale=g3[:, 0:1])
                        nc.vector.scalar_tensor_tensor(
                            out=o[:], in0=Hcat[:, OUT_DIM:D2], scalar=g3[:, 1:2],
                            in1=o[:], op0=OP.mult, op1=OP.add)
                        nc.vector.scalar_tensor_tensor(
                            out=o[:], in0=Hcat[:, D2:DCAT], scalar=g3[:, 2:3],
                            in1=o[:], op0=OP.mult, op1=OP.add)
                        mx = ep.tile([P, 4], f32, tag="mx")
                        nc.vector.tensor_reduce(out=mx[:, 0:1], in_=o[:],
                                                op=OP.max,
                                                axis=mybir.AxisListType.X,
                                                negate=True)
                        et = ep.tile([P, OUT_DIM], f32, tag="et")
                        nc.scalar.activation(out=et[:], in_=o[:], func=AF.Exp,
                                             bias=mx[:, 0:1], scale=1.0,
                                             accum_out=mx[:, 1:2])
                        nc.scalar.activation(out=mx[:, 2:3], in_=mx[:, 1:2],
                                             func=AF.Ln)
                        fin = ep.tile([P, OUT_DIM], f32, tag="fin")
                        nc.vector.tensor_scalar(
                            out=fin[:], in0=o[:], scalar1=mx[:, 0:1],
                            scalar2=mx[:, 2:3], op0=OP.add, op1=OP.subtract)
                        nc.sync.dma_start(out=out_ext[t * P:(t + 1) * P, :],
                                          in_=fin[:])

    nc.compile()
    return nc


def kernel(**inputs) -> np.ndarray:
    consts, meta, per_core = _build_host_data(**inputs)
    nc = _build_bass(meta)

    in_maps = []
    for c in range(NCORES):
        in_maps.append(dict(
            xT=np.ascontiguousarray(per_core["xT"][c]),
            disq=np.ascontiguousarray(per_core["disq_col"][c]),
            idx=np.ascontiguousarray(per_core["idx_wrapped"][c]),
            sel=per_core["sel_stream"][c].reshape(P, -1),
            Wcat=consts["Wcat"], bias_rep=consts["bias_rep"],
            wg_rep=consts["wg_rep"], bg_rep=consts["bg_rep"],
        ))

    from concourse.bass_utils import run_bass_kernel_spmd
    res = run_bass_kernel_spmd(nc, in_maps, core_ids=list(range(NCORES)))
    out = np.concatenate([res.results[c]["out"][:SHARD] for c in range(NCORES)],
                         axis=0)
    return out.astype(np.float32)


if __name__ == "__main__":
    import reference
    ins = {k: np.asarray(v) for k, v in reference.setup_inputs().items()}
    got = kernel(**ins)
    exp = np.asarray(reference.reference(**reference.setup_inputs()))
    rel = np.linalg.norm(got - exp) / np.linalg.norm(exp)
    print("Relative error:", rel)


# revision 32
# speedup vs baseline: 1.1014x; 1.0473x over previous
"""ACM-GCN single-layer kernel for Trainium2, 8 NeuronCores (SPMD).

Strategy (graph/data parallel):
- Nodes partitioned 12500/core (padded to 12544 = 98*128).
- Phase 1: h = x_shard @ [W_hp|W_lp|W_i] + b (bf16 PE matmul); h_hp/h_i kept
  in SBUF; deg_isqrt-prescaled bf16 table [12544, 128] written to DRAM.
- AllGather the table -> full [100352, 128] bf16 table per core.
- Phase 2: per dst tile, dma_gather the source rows of its edges (4 SWDGE
  queues, int16 indices via 4 source buckets), build one-hot selection
  matrices (tensor_scalar is_equal vs iota -> DVE 4x mode) and accumulate
  sel.T @ gathered in PSUM.  Self-edges appended so the self-loop term is
  folded into the aggregation.  Epilogue fuses HP/LP/I branches, gates and
  log_softmax.
"""
import numpy as np
import ml_dtypes

N_NODES = 100000
N_EDGES = 3200000
IN_DIM = 256
OUT_DIM = 64
NCORES = 8
P = 128
SHARD = N_NODES // NCORES            # 12500
NT = (SHARD + P - 1) // P            # 98 tiles
SHARD_PAD = NT * P                   # 12544
NTOT_PAD = SHARD_PAD * NCORES        # 100352
NBUCK = 4
BUCK = NTOT_PAD // NBUCK             # 25088 (< 32768, int16-safe)
D2 = 2 * OUT_DIM                     # 128 gathered feature dim (hp|lp)
DCAT = 3 * OUT_DIM                   # 192
SUPER = 2                            # dst tiles per gather group
HNT = NT // 2                        # 49 tiles per half
HALF = HNT * P                       # 6272 rows per half
BF16 = ml_dtypes.bfloat16


def _build_host_data(x, edge_index, W_hp, b_hp, W_lp, b_lp, W_i, b_i,
                     w_gh, b_gh, w_gl, b_gl, w_gi, b_gi):
    src = np.asarray(edge_index[0], dtype=np.int64)
    dst = np.asarray(edge_index[1], dtype=np.int64)

    deg = np.bincount(dst, minlength=N_NODES).astype(np.float64) + 1.0
    disqrt = (1.0 / np.sqrt(deg)).astype(np.float32)

    # self edges: with the prescaled table they contribute deg_inv * h
    allv = np.arange(N_NODES, dtype=np.int64)
    src = np.concatenate([src, allv])
    dst = np.concatenate([dst, allv])

    core = dst // SHARD
    dloc = dst - core * SHARD
    tile = dloc // P
    dstloc = (dloc % P).astype(np.int32)
    # half-major padded global index: the table is all-gathered as two
    # per-half collectives, so global row = half*8*HALF + core*HALF + loc%HALF
    s_core = src // SHARD
    s_loc = src % SHARD
    s_half = s_loc // HALF
    spad = s_half * (NCORES * HALF) + s_core * HALF + (s_loc - s_half * HALF)
    buck = (spad // BUCK).astype(np.int32)
    idx16 = (spad - buck.astype(np.int64) * BUCK).astype(np.int16)

    key = ((core * NT + tile) * NBUCK + buck).astype(np.int64)
    order = np.argsort(key, kind="stable")
    idx16_s = idx16[order]
    dstloc_s = dstloc[order]
    cnts = np.bincount(key[order], minlength=NCORES * NT * NBUCK).reshape(
        NCORES, NT, NBUCK)

    NGRP = (NT + SUPER - 1) // SUPER
    # pack each (group, bucket)'s edges contiguously (tiles concatenated, one
    # pad-to-128 at the end); dl encodes tile-within-group*128 + dstloc
    grp_cnt = np.zeros((NCORES, NGRP, NBUCK), np.int64)
    for g in range(NGRP):
        ts = list(range(g * SUPER, min((g + 1) * SUPER, NT)))
        grp_cnt[:, g, :] = cnts[:, ts, :].sum(axis=1)
    C_gb = np.ceil(grp_cnt.max(axis=0) / P).astype(np.int64)       # [NGRP, NBUCK]
    NCHUNK_TOT = int(C_gb.sum())
    NIDX_TOT = NCHUNK_TOT * P
    grp_nidx = C_gb * P

    core_seg_start = np.cumsum(cnts.reshape(NCORES, -1), axis=1).reshape(
        NCORES, NT, NBUCK) - cnts
    core_base = np.concatenate([[0], np.cumsum(cnts.sum(axis=(1, 2)))])[:-1]

    idx_stream = np.zeros((NCORES, NIDX_TOT), np.int16)
    dl_stream = np.full((NCORES, NIDX_TOT), 300.0, np.float32)
    pos = 0
    for g in range(NGRP):
        ts = list(range(g * SUPER, min((g + 1) * SUPER, NT)))
        for b in range(NBUCK):
            n_pad = int(C_gb[g, b]) * P
            if n_pad == 0:
                continue
            for c in range(NCORES):
                p0 = pos
                for ti, t in enumerate(ts):
                    s0 = core_base[c] + core_seg_start[c, t, b]
                    n = int(cnts[c, t, b])
                    idx_stream[c, p0:p0 + n] = idx16_s[s0:s0 + n]
                    dl_stream[c, p0:p0 + n] = dstloc_s[s0:s0 + n] + ti * P
                    p0 += n
            pos += n_pad
    assert pos == NIDX_TOT

    idx_wrapped = np.zeros((NCORES, 128, NIDX_TOT // 16), np.int16)
    for c in range(NCORES):
        a = idx_stream[c].reshape(NIDX_TOT // 16, 16).T
        idx_wrapped[c] = np.tile(a, (8, 1))

    # matmul schedule: per (g, b, chunk) the set of group-local tiles it can
    # touch on any core; one streamed one-hot sel block per (chunk, tile) pair
    mm_sched = []          # (g, b, k, ti)
    grp_nmm = np.zeros(NGRP, np.int64)
    for g in range(NGRP):
        ts = list(range(g * SUPER, min((g + 1) * SUPER, NT)))
        nmm = 0
        for b in range(NBUCK):
            run_min = np.zeros(len(ts) + 1, np.int64)
            run_max = np.zeros(len(ts) + 1, np.int64)
            for ti, t in enumerate(ts):
                run_min[ti + 1] = run_min[ti] + cnts[:, t, b].min()
                run_max[ti + 1] = run_max[ti] + cnts[:, t, b].max()
            for k in range(int(C_gb[g, b])):
                ks, ke = k * P, (k + 1) * P
                for ti, t in enumerate(ts):
                    if ke > run_min[ti] and ks < run_max[ti + 1]:
                        mm_sched.append((g, b, k, ti))
                        nmm += 1
        grp_nmm[g] = nmm
    NMM_TOT = len(mm_sched)

    # chunk start offsets in the idx/dl stream, per (g, b)
    gb_off = {}
    off = 0
    for g in range(NGRP):
        for b in range(NBUCK):
            gb_off[(g, b)] = off
            off += int(C_gb[g, b]) * P

    ONE = np.float32(1.0).astype(BF16).view(np.uint16)
    sel_stream = np.zeros((NCORES, 128, NMM_TOT * P), np.uint16)
    dvals = np.arange(P, dtype=np.float32)
    for m, (g, b, k, ti) in enumerate(mm_sched):
        base = gb_off[(g, b)] + k * P
        for c in range(NCORES):
            col = dl_stream[c, base:base + P] - ti * P       # [128 edges]
            mask = (col >= 0) & (col < P)
            pp = np.nonzero(mask)[0]
            sel_stream[c, pp, m * P + col[pp].astype(np.int64)] = ONE
    sel_stream = sel_stream.view(BF16)

    W_cat = np.concatenate([W_hp, W_lp, W_i], axis=1).astype(np.float32)
    b_cat = np.concatenate([b_hp, b_lp, b_i]).astype(np.float32)
    wg_cat = np.concatenate([w_gh[:, 0], w_gl[:, 0], w_gi[:, 0]]).astype(np.float32)
    bg_cat = np.array([b_gh[0], b_gl[0], b_gi[0]], np.float32)

    xT = np.zeros((NCORES, IN_DIM, SHARD_PAD), BF16)
    disq_col = np.ones((NCORES, P, NT), np.float32)
    x = np.asarray(x, np.float32)
    for c in range(NCORES):
        xT[c, :, :SHARD] = x[c * SHARD:(c + 1) * SHARD].T.astype(BF16)
        d = np.ones(SHARD_PAD, np.float32)
        d[:SHARD] = disqrt[c * SHARD:(c + 1) * SHARD]
        disq_col[c] = d.reshape(NT, P).T

    consts = dict(
        Wcat=W_cat.astype(BF16),
        bias_rep=np.tile(b_cat[None, :], (P, 1)).astype(np.float32),
        wg_rep=np.tile(wg_cat[None, :], (P, 1)).astype(np.float32),
        bg_rep=np.tile(bg_cat[None, :], (P, 1)).astype(np.float32),
        iota=np.tile(np.arange(SUPER * P, dtype=np.float32)[None, :],
                     (P, 1)).astype(BF16),
    )
    meta = dict(C_gb=C_gb, grp_cnt=grp_cnt, cnts=cnts, grp_nidx=grp_nidx,
                NCHUNK_TOT=NCHUNK_TOT, NIDX_TOT=NIDX_TOT, NGRP=NGRP,
                mm_sched=mm_sched, grp_nmm=grp_nmm, NMM_TOT=NMM_TOT)
    per_core = dict(xT=xT, disq_col=disq_col, idx_wrapped=idx_wrapped,
                    sel_stream=sel_stream)
    return consts, meta, per_core


def _force_act_set():
    """Make every activation use the one table set that holds relu+exp+ln+copy
    (index preserved), so the kernel loads the ACT table exactly once."""
    import concourse.hw_specs as hw_specs
    if getattr(hw_specs, "_acm_patched", False):
        return
    orig = hw_specs.get_activation_tables

    def patched(module_arch):
        tabs = orig(module_arch)
        full = None
        for name, funcs in tabs.items():
            fn = {str(f) for f in funcs}
            if any("Exp" in f for f in fn) and any("Ln" in str(f) for f in fn) \
               and any("Relu" in f for f in fn):
                full = name
                break
        if full is None:
            return tabs
        keep = tabs[full]
        return {name: (funcs if name == full else (funcs & keep) - keep)
                for name, funcs in tabs.items()}

    hw_specs.get_activation_tables = patched
    import concourse.bacc as bacc_mod
    bacc_mod.get_activation_tables = patched
    hw_specs._acm_patched = True


def _build_bass(meta):
    import concourse.bacc as bacc
    import concourse.tile as tile
    from concourse import mybir

    _force_act_set()

    C_gb = meta["C_gb"]
    grp_cnt = meta["grp_cnt"]
    cnts = meta["cnts"]
    grp_nidx = meta["grp_nidx"]
    NCHUNK_TOT = meta["NCHUNK_TOT"]
    NIDX_TOT = meta["NIDX_TOT"]
    NGRP = meta["NGRP"]
    mm_sched = meta["mm_sched"]
    grp_nmm = meta["grp_nmm"]
    NMM_TOT = meta["NMM_TOT"]

    nc = bacc.Bacc("TRN2", target_bir_lowering=False, debug=False,
                   num_devices=NCORES, num_swdge_queues=4)

    f32, bf16, i16 = mybir.dt.float32, mybir.dt.bfloat16, mybir.dt.int16
    AF = mybir.ActivationFunctionType
    OP = mybir.AluOpType

    xT_in = nc.dram_tensor("xT", [IN_DIM, SHARD_PAD], bf16, kind="ExternalInput")
    disq_in = nc.dram_tensor("disq", [P, NT], f32, kind="ExternalInput")
    idx_in = nc.dram_tensor("idx", [P, NIDX_TOT // 16], i16, kind="ExternalInput")
    sel_in = nc.dram_tensor("sel", [P, NMM_TOT * P], bf16, kind="ExternalInput")
    Wcat_in = nc.dram_tensor("Wcat", [IN_DIM, DCAT], bf16, kind="ExternalInput")
    bias_in = nc.dram_tensor("bias_rep", [P, DCAT], f32, kind="ExternalInput")
    wg_in = nc.dram_tensor("wg_rep", [P, DCAT], f32, kind="ExternalInput")
    bg_in = nc.dram_tensor("bg_rep", [P, 3], f32, kind="ExternalInput")
    out_ext = nc.dram_tensor("out", [SHARD_PAD, OUT_DIM], f32, kind="ExternalOutput")

    table_own_a = nc.dram_tensor("table_own_a", [HALF, D2], bf16)
    table_own_b = nc.dram_tensor("table_own_b", [SHARD_PAD - HALF, D2], bf16)
    wu_src = nc.dram_tensor("wu_src", [128, D2], bf16)
    wu_cin = nc.dram_tensor("wu_cin", [128, 8], bf16)
    wu_cout = nc.dram_tensor("wu_cout", [NCORES * 128, 8], bf16, addr_space="Shared")
    table_full_a = nc.dram_tensor("table_full_a", [NCORES * HALF, D2], bf16,
                                  addr_space="Shared")
    table_full_b = nc.dram_tensor("table_full_b", [NTOT_PAD - NCORES * HALF, D2],
                                  bf16, addr_space="Shared")

    with tile.TileContext(nc) as tc:
        with (
            tc.tile_pool(name="consts", bufs=1) as consts,
            tc.tile_pool(name="hpool", bufs=1) as hpool,
        ):
            # --- warmups: first collective and first SWDGE gather are cold
            # (ncfw staging / Q7 library load); run tiny dummies early so the
            # real ones hit warm paths, overlapped with phase 1 ---
            wu_idx = consts.tile([P, 8], mybir.dt.int16)
            nc.gpsimd.memset(wu_idx[:], 0)
            wu_out = consts.tile([P, 1, D2], bf16)
            nc.gpsimd.dma_gather(
                out_ap=wu_out[:], in_ap=wu_src[:], idxs_ap=wu_idx[:],
                num_idxs=128, num_idxs_reg=128, elem_size=D2,
                single_packet=False, queue_num=0)
            nc.gpsimd.collective_compute(
                "AllGather", mybir.AluOpType.bypass,
                replica_groups=[list(range(NCORES))],
                ins=[wu_cin[:]], outs=[wu_cout[:]],
            )
            W0 = consts.tile([P, DCAT], bf16)
            W1 = consts.tile([P, DCAT], bf16)
            nc.sync.dma_start(out=W0[:], in_=Wcat_in[0:P, :])
            nc.sync.dma_start(out=W1[:], in_=Wcat_in[P:2 * P, :])
            bias_t = consts.tile([P, DCAT], f32)
            nc.sync.dma_start(out=bias_t[:], in_=bias_in[:])
            wg_t = consts.tile([P, DCAT], f32)
            nc.sync.dma_start(out=wg_t[:], in_=wg_in[:])
            bg_t = consts.tile([P, 3], f32)
            nc.sync.dma_start(out=bg_t[:], in_=bg_in[:])
            disq_t = consts.tile([P, NT], f32)
            nc.sync.dma_start(out=disq_t[:], in_=disq_in[:])
            ndisq_t = consts.tile([P, NT], f32)
            nc.vector.tensor_scalar_mul(out=ndisq_t[:], in0=disq_t[:], scalar1=-1.0)

            # h_slab holds [h_hp | h_i] per tile: [128, 98*128] bf16
            h_slab = hpool.tile([P, NT * D2], bf16)

            # ---- phase 1 ----
            with (
                tc.tile_pool(name="xt", bufs=1) as xtp,
                tc.tile_pool(name="p1", bufs=3) as p1,
                tc.tile_pool(name="p1ps", bufs=2, space="PSUM") as p1ps,
            ):
                xt0 = xtp.tile([P, SHARD_PAD], bf16)
                xt1 = xtp.tile([P, SHARD_PAD], bf16)
                QW = SHARD_PAD // 4
                for qi in range(4):
                    nc.sync.dma_start(out=xt0[:, qi * QW:(qi + 1) * QW],
                                      in_=xT_in[0:P, qi * QW:(qi + 1) * QW])
                    nc.sync.dma_start(out=xt1[:, qi * QW:(qi + 1) * QW],
                                      in_=xT_in[P:2 * P, qi * QW:(qi + 1) * QW])
                for t in range(NT):
                    ps = p1ps.tile([P, DCAT], f32, tag="p1ps")
                    nc.tensor.matmul(out=ps[:], lhsT=xt0[:, t * P:(t + 1) * P],
                                     rhs=W0[:], start=True, stop=False)
                    nc.tensor.matmul(out=ps[:], lhsT=xt1[:, t * P:(t + 1) * P],
                                     rhs=W1[:], start=False, stop=True)
                    sc = p1.tile([P, DCAT], f32, tag="sc")
                    nc.vector.tensor_add(out=sc[:], in0=ps[:], in1=bias_t[:])
                    # h_slab tile t = [hp (0:64) | i (128:192)]
                    sc3 = sc[:].rearrange("p (a b) -> p a b", a=3)
                    nc.vector.tensor_copy(
                        out=h_slab[:, t * D2:(t + 1) * D2].rearrange(
                            "p (a b) -> p a b", a=2),
                        in_=sc3[:, 0::2, :])
                    tab = p1.tile([P, D2], bf16, tag="tab")
                    nc.scalar.activation(out=tab[:], in_=sc[:, 0:D2],
                                         func=AF.Copy, scale=disq_t[:, t:t + 1])
                    if t < HNT:
                        nc.sync.dma_start(out=table_own_a[t * P:(t + 1) * P, :],
                                          in_=tab[:])
                    else:
                        tb = t - HNT
                        nc.sync.dma_start(out=table_own_b[tb * P:(tb + 1) * P, :],
                                          in_=tab[:])
                    if t == HNT - 1:
                        nc.gpsimd.collective_compute(
                            "AllGather", mybir.AluOpType.bypass,
                            replica_groups=[list(range(NCORES))],
                            ins=[table_own_a[:]], outs=[table_full_a[:]],
                        )

            nc.gpsimd.collective_compute(
                "AllGather", mybir.AluOpType.bypass,
                replica_groups=[list(range(NCORES))],
                ins=[table_own_b[:]], outs=[table_full_b[:]],
            )

            # ---- phase 2 ----
            with (
                tc.tile_pool(name="gath", bufs=4) as gpool,
                tc.tile_pool(name="idxp", bufs=6) as idxp,
                tc.tile_pool(name="sel", bufs=2) as selp,
                tc.tile_pool(name="ep", bufs=3) as ep,
                tc.tile_pool(name="ps2", bufs=8, space="PSUM") as ps2,
            ):
                idx_off = 0
                mm_off = 0
                qn = 0
                max_nmm = int(grp_nmm.max())
                for g in range(NGRP):
                    ts = list(range(g * SUPER, min((g + 1) * SUPER, NT)))
                    nidx_g = int(grp_nidx[g].sum())
                    if nidx_g == 0:
                        continue
                    icols = nidx_g // 16
                    idx_t = idxp.tile([P, icols], i16, tag="idx")
                    nc.sync.dma_start(
                        out=idx_t[:],
                        in_=idx_in[:, idx_off // 16:idx_off // 16 + icols])
                    gts = []
                    off_in_g = 0
                    for b in range(NBUCK):
                        nb = int(grp_nidx[g, b])
                        if nb == 0:
                            gts.append(None)
                            continue
                        gt = gpool.tile([P, nb // P, D2], bf16, tag=f"g{b}")
                        if b < 2:
                            src_tab = table_full_a[b * BUCK:(b + 1) * BUCK, :]
                        else:
                            src_tab = table_full_b[(b - 2) * BUCK:(b - 1) * BUCK, :]
                        nc.gpsimd.dma_gather(
                            out_ap=gt[:],
                            in_ap=src_tab,
                            idxs_ap=idx_t[:, off_in_g // 16:(off_in_g + nb) // 16],
                            num_idxs=nb, num_idxs_reg=nb, elem_size=D2,
                            single_packet=False, queue_num=qn,
                        )
                        qn = (qn + 1) % 4
                        gts.append(gt)
                        off_in_g += nb
                    idx_off += nidx_g

                    nmm_g = int(grp_nmm[g])
                    selst = selp.tile([P, max_nmm, P], bf16, tag="selst")
                    sched_g = mm_sched[mm_off:mm_off + nmm_g]
                    nc.sync.dma_start(
                        out=selst[:, 0:nmm_g, :],
                        in_=sel_in[:, mm_off * P:(mm_off + nmm_g) * P])
                    n_touch = {t: 0 for t in ts}
                    for (gg, b, k, ti) in sched_g:
                        n_touch[ts[ti]] += 1
                    psums = {}
                    first = {t: True for t in ts}
                    done = {t: 0 for t in ts}
                    for m, (gg, b, k, ti) in enumerate(sched_g):
                        t = ts[ti]
                        if t not in psums:
                            psums[t] = ps2.tile([P, D2], f32, tag="acc",
                                                name=f"acc{t}")
                        done[t] += 1
                        nc.tensor.matmul(
                            out=psums[t][:], lhsT=selst[:, m, :],
                            rhs=gts[b][:, k, :],
                            start=first[t],
                            stop=(done[t] == n_touch[t]))
                        first[t] = False
                    mm_off += nmm_g

                    for t in ts:
                        acc = psums[t]
                        hof = t * D2
                        Hcat = ep.tile([P, DCAT], f32, tag="Hcat")
                        nc.vector.scalar_tensor_tensor(
                            out=Hcat[:, 0:OUT_DIM], in0=acc[:, 0:OUT_DIM],
                            scalar=ndisq_t[:, t:t + 1],
                            in1=h_slab[:, hof:hof + OUT_DIM],
                            op0=OP.mult, op1=OP.add)
                        nc.scalar.activation(out=Hcat[:, 0:OUT_DIM],
                                             in_=Hcat[:, 0:OUT_DIM], func=AF.Relu)
                        nc.scalar.activation(out=Hcat[:, OUT_DIM:D2],
                                             in_=acc[:, OUT_DIM:D2], func=AF.Relu,
                                             scale=disq_t[:, t:t + 1])
                        nc.scalar.activation(out=Hcat[:, D2:DCAT],
                                             in_=h_slab[:, hof + OUT_DIM:hof + D2],
                                             func=AF.Relu)
                        gm = ep.tile([P, DCAT], f32, tag="gm")
                        nc.vector.tensor_tensor(out=gm[:], in0=Hcat[:], in1=wg_t[:],
                                                op=OP.mult)
                        g3 = ep.tile([P, 4], f32, tag="g3")
                        nc.vector.reduce_sum(
                            out=g3[:, 0:3],
                            in_=gm[:].rearrange("p (a b) -> p a b", a=3),
                            axis=mybir.AxisListType.X)
                        nc.vector.tensor_add(out=g3[:, 0:3], in0=g3[:, 0:3],
                                             in1=bg_t[:])
                        o = ep.tile([P, OUT_DIM], f32, tag="o")
                        nc.scalar.activation(out=o[:], in_=Hcat[:, 0:OUT_DIM],
                                             func=AF.Copy, scale=g3[:, 0:1])
                        nc.vector.scalar_tensor_tensor(
                            out=o[:], in0=Hcat[:, OUT_DIM:D2], scalar=g3[:, 1:2],
                            in1=o[:], op0=OP.mult, op1=OP.add)
                        nc.vector.scalar_tensor_tensor(
                            out=o[:], in0=Hcat[:, D2:DCAT], scalar=g3[:, 2:3],
                            in1=o[:], op0=OP.mult, op1=OP.add)
                        mx = ep.tile([P, 4], f32, tag="mx")
                        nc.vector.tensor_reduce(out=mx[:, 0:1], in_=o[:],
                                                op=OP.max,
                                                axis=mybir.AxisListType.X,
                                                negate=True)
                        et = ep.tile([P, OUT_DIM], f32, tag="et")
                        nc.scalar.activation(out=et[:], in_=o[:], func=AF.Exp,
                                             bias=mx[:, 0:1], scale=1.0,
                                             accum_out=mx[:, 1:2])
                        nc.scalar.activation(out=mx[:, 2:3], in_=mx[:, 1:2],
                                             func=AF.Ln)
                        fin = ep.tile([P, OUT_DIM], f32, tag="fin")
                        nc.vector.tensor_scalar(
                            out=fin[:], in0=o[:], scalar1=mx[:, 0:1],
                            scalar2=mx[:, 2:3], op0=OP.add, op1=OP.subtract)
                        nc.sync.dma_start(out=out_ext[t * P:(t + 1) * P, :],
                                          in_=fin[:])

    nc.compile()
    return nc


def kernel(**inputs) -> np.ndarray:
    consts, meta, per_core = _build_host_data(**inputs)
    nc = _build_bass(meta)

    in_maps = []
    for c in range(NCORES):
        in_maps.append(dict(
            xT=np.ascontiguousarray(per_core["xT"][c]),
            disq=np.ascontiguousarray(per_core["disq_col"][c]),
            idx=np.ascontiguousarray(per_core["idx_wrapped"][c]),
            sel=per_core["sel_stream"][c].reshape(P, -1),
            Wcat=consts["Wcat"], bias_rep=consts["bias_rep"],
            wg_rep=consts["wg_rep"], bg_rep=consts["bg_rep"],
        ))

    from concourse.bass_utils import run_bass_kernel_spmd
    res = run_bass_kernel_spmd(nc, in_maps, core_ids=list(range(NCORES)))
    out = np.concatenate([res.results[c]["out"][:SHARD] for c in range(NCORES)],
                         axis=0)
    return out.astype(np.float32)


if __name__ == "__main__":
    import reference
    ins = {k: np.asarray(v) for k, v in reference.setup_inputs().items()}
    got = kernel(**ins)
    exp = np.asarray(reference.reference(**reference.setup_inputs()))
    rel = np.linalg.norm(got - exp) / np.linalg.norm(exp)
    print("Relative error:", rel)


# revision 33
# speedup vs baseline: 1.2653x; 1.1488x over previous
"""ACM-GCN single-layer kernel for Trainium2, 8 NeuronCores (SPMD).

Strategy (graph/data parallel):
- Nodes partitioned 12500/core (padded to 12544 = 98*128).
- Phase 1: h = x_shard @ [W_hp|W_lp|W_i] + b (bf16 PE matmul); h_hp/h_i kept
  in SBUF; deg_isqrt-prescaled bf16 table [12544, 128] written to DRAM.
- AllGather the table -> full [100352, 128] bf16 table per core.
- Phase 2: per dst tile, dma_gather the source rows of its edges (4 SWDGE
  queues, int16 indices via 4 source buckets), build one-hot selection
  matrices (tensor_scalar is_equal vs iota -> DVE 4x mode) and accumulate
  sel.T @ gathered in PSUM.  Self-edges appended so the self-loop term is
  folded into the aggregation.  Epilogue fuses HP/LP/I branches, gates and
  log_softmax.
"""
import numpy as np
import ml_dtypes

N_NODES = 100000
N_EDGES = 3200000
IN_DIM = 256
OUT_DIM = 64
NCORES = 8
P = 128
SHARD = N_NODES // NCORES            # 12500
NT = (SHARD + P - 1) // P            # 98 tiles
SHARD_PAD = NT * P                   # 12544
NTOT_PAD = SHARD_PAD * NCORES        # 100352
NBUCK = 4
BUCK = NTOT_PAD // NBUCK             # 25088 (< 32768, int16-safe)
D2 = 2 * OUT_DIM                     # 128 gathered feature dim (hp|lp)
DCAT = 3 * OUT_DIM                   # 192
SUPER = 2                            # dst tiles per gather group
HNT = NT // 2                        # 49 tiles per half
HALF = HNT * P                       # 6272 rows per half
BF16 = ml_dtypes.bfloat16


def _build_host_data(x, edge_index, W_hp, b_hp, W_lp, b_lp, W_i, b_i,
                     w_gh, b_gh, w_gl, b_gl, w_gi, b_gi):
    src = np.asarray(edge_index[0], dtype=np.int64)
    dst = np.asarray(edge_index[1], dtype=np.int64)

    deg = np.bincount(dst, minlength=N_NODES).astype(np.float64) + 1.0
    disqrt = (1.0 / np.sqrt(deg)).astype(np.float32)

    # self edges: with the prescaled table they contribute deg_inv * h
    allv = np.arange(N_NODES, dtype=np.int64)
    src = np.concatenate([src, allv])
    dst = np.concatenate([dst, allv])

    core = dst // SHARD
    dloc = dst - core * SHARD
    tile = dloc // P
    dstloc = (dloc % P).astype(np.int32)
    # half-major padded global index: the table is all-gathered as two
    # per-half collectives, so global row = half*8*HALF + core*HALF + loc%HALF
    s_core = src // SHARD
    s_loc = src % SHARD
    s_half = s_loc // HALF
    spad = s_half * (NCORES * HALF) + s_core * HALF + (s_loc - s_half * HALF)
    buck = (spad // BUCK).astype(np.int32)
    idx16 = (spad - buck.astype(np.int64) * BUCK).astype(np.int16)

    key = ((core * NT + tile) * NBUCK + buck).astype(np.int64)
    order = np.argsort(key, kind="stable")
    idx16_s = idx16[order]
    dstloc_s = dstloc[order]
    cnts = np.bincount(key[order], minlength=NCORES * NT * NBUCK).reshape(
        NCORES, NT, NBUCK)

    NGRP = (NT + SUPER - 1) // SUPER
    # pack each (group, bucket)'s edges contiguously (tiles concatenated, one
    # pad-to-128 at the end); dl encodes tile-within-group*128 + dstloc
    grp_cnt = np.zeros((NCORES, NGRP, NBUCK), np.int64)
    for g in range(NGRP):
        ts = list(range(g * SUPER, min((g + 1) * SUPER, NT)))
        grp_cnt[:, g, :] = cnts[:, ts, :].sum(axis=1)
    C_gb = np.ceil(grp_cnt.max(axis=0) / P).astype(np.int64)       # [NGRP, NBUCK]
    NCHUNK_TOT = int(C_gb.sum())
    NIDX_TOT = NCHUNK_TOT * P
    grp_nidx = C_gb * P

    core_seg_start = np.cumsum(cnts.reshape(NCORES, -1), axis=1).reshape(
        NCORES, NT, NBUCK) - cnts
    core_base = np.concatenate([[0], np.cumsum(cnts.sum(axis=(1, 2)))])[:-1]

    idx_stream = np.zeros((NCORES, NIDX_TOT), np.int16)
    dl_stream = np.full((NCORES, NIDX_TOT), 300.0, np.float32)
    pos = 0
    for g in range(NGRP):
        ts = list(range(g * SUPER, min((g + 1) * SUPER, NT)))
        for b in range(NBUCK):
            n_pad = int(C_gb[g, b]) * P
            if n_pad == 0:
                continue
            for c in range(NCORES):
                p0 = pos
                for ti, t in enumerate(ts):
                    s0 = core_base[c] + core_seg_start[c, t, b]
                    n = int(cnts[c, t, b])
                    idx_stream[c, p0:p0 + n] = idx16_s[s0:s0 + n]
                    dl_stream[c, p0:p0 + n] = dstloc_s[s0:s0 + n] + ti * P
                    p0 += n
            pos += n_pad
    assert pos == NIDX_TOT

    idx_wrapped = np.zeros((NCORES, 128, NIDX_TOT // 16), np.int16)
    for c in range(NCORES):
        a = idx_stream[c].reshape(NIDX_TOT // 16, 16).T
        idx_wrapped[c] = np.tile(a, (8, 1))

    # matmul schedule: per (g, b, chunk) the set of group-local tiles it can
    # touch on any core; one streamed one-hot sel block per (chunk, tile) pair
    mm_sched = []          # (g, b, k, ti)
    grp_nmm = np.zeros(NGRP, np.int64)
    for g in range(NGRP):
        ts = list(range(g * SUPER, min((g + 1) * SUPER, NT)))
        nmm = 0
        for b in range(NBUCK):
            run_min = np.zeros(len(ts) + 1, np.int64)
            run_max = np.zeros(len(ts) + 1, np.int64)
            for ti, t in enumerate(ts):
                run_min[ti + 1] = run_min[ti] + cnts[:, t, b].min()
                run_max[ti + 1] = run_max[ti] + cnts[:, t, b].max()
            for k in range(int(C_gb[g, b])):
                ks, ke = k * P, (k + 1) * P
                for ti, t in enumerate(ts):
                    if ke > run_min[ti] and ks < run_max[ti + 1]:
                        mm_sched.append((g, b, k, ti))
                        nmm += 1
        grp_nmm[g] = nmm
    NMM_TOT = len(mm_sched)

    # chunk start offsets in the idx/dl stream, per (g, b)
    gb_off = {}
    off = 0
    for g in range(NGRP):
        for b in range(NBUCK):
            gb_off[(g, b)] = off
            off += int(C_gb[g, b]) * P

    ONE = np.float32(1.0).astype(ml_dtypes.float8_e4m3).view(np.uint8)
    sel_stream = np.zeros((NCORES, 128, NMM_TOT * P), np.uint8)
    dvals = np.arange(P, dtype=np.float32)
    for m, (g, b, k, ti) in enumerate(mm_sched):
        base = gb_off[(g, b)] + k * P
        for c in range(NCORES):
            col = dl_stream[c, base:base + P] - ti * P       # [128 edges]
            mask = (col >= 0) & (col < P)
            pp = np.nonzero(mask)[0]
            sel_stream[c, pp, m * P + col[pp].astype(np.int64)] = ONE
    sel_stream = sel_stream.view(ml_dtypes.float8_e4m3)

    W_cat = np.concatenate([W_hp, W_lp, W_i], axis=1).astype(np.float32)
    b_cat = np.concatenate([b_hp, b_lp, b_i]).astype(np.float32)
    wg_cat = np.concatenate([w_gh[:, 0], w_gl[:, 0], w_gi[:, 0]]).astype(np.float32)
    bg_cat = np.array([b_gh[0], b_gl[0], b_gi[0]], np.float32)

    xT = np.zeros((NCORES, IN_DIM, SHARD_PAD), BF16)
    disq_col = np.ones((NCORES, P, NT), np.float32)
    x = np.asarray(x, np.float32)
    for c in range(NCORES):
        xT[c, :, :SHARD] = x[c * SHARD:(c + 1) * SHARD].T.astype(BF16)
        d = np.ones(SHARD_PAD, np.float32)
        d[:SHARD] = disqrt[c * SHARD:(c + 1) * SHARD]
        disq_col[c] = d.reshape(NT, P).T

    consts = dict(
        Wcat=W_cat.astype(BF16),
        bias_rep=np.tile(b_cat[None, :], (P, 1)).astype(np.float32),
        wg_rep=np.tile(wg_cat[None, :], (P, 1)).astype(np.float32),
        bg_rep=np.tile(bg_cat[None, :], (P, 1)).astype(np.float32),
        iota=np.tile(np.arange(SUPER * P, dtype=np.float32)[None, :],
                     (P, 1)).astype(BF16),
    )
    meta = dict(C_gb=C_gb, grp_cnt=grp_cnt, cnts=cnts, grp_nidx=grp_nidx,
                NCHUNK_TOT=NCHUNK_TOT, NIDX_TOT=NIDX_TOT, NGRP=NGRP,
                mm_sched=mm_sched, grp_nmm=grp_nmm, NMM_TOT=NMM_TOT)
    per_core = dict(xT=xT, disq_col=disq_col, idx_wrapped=idx_wrapped,
                    sel_stream=sel_stream)
    return consts, meta, per_core


def _force_act_set():
    """Make every activation use the one table set that holds relu+exp+ln+copy
    (index preserved), so the kernel loads the ACT table exactly once."""
    import concourse.hw_specs as hw_specs
    if getattr(hw_specs, "_acm_patched", False):
        return
    orig = hw_specs.get_activation_tables

    def patched(module_arch):
        tabs = orig(module_arch)
        full = None
        for name, funcs in tabs.items():
            fn = {str(f) for f in funcs}
            if any("Exp" in f for f in fn) and any("Ln" in str(f) for f in fn) \
               and any("Relu" in f for f in fn):
                full = name
                break
        if full is None:
            return tabs
        keep = tabs[full]
        return {name: (funcs if name == full else (funcs & keep) - keep)
                for name, funcs in tabs.items()}

    hw_specs.get_activation_tables = patched
    import concourse.bacc as bacc_mod
    bacc_mod.get_activation_tables = patched
    hw_specs._acm_patched = True


def _build_bass(meta):
    import concourse.bacc as bacc
    import concourse.tile as tile
    from concourse import mybir

    _force_act_set()

    C_gb = meta["C_gb"]
    grp_cnt = meta["grp_cnt"]
    cnts = meta["cnts"]
    grp_nidx = meta["grp_nidx"]
    NCHUNK_TOT = meta["NCHUNK_TOT"]
    NIDX_TOT = meta["NIDX_TOT"]
    NGRP = meta["NGRP"]
    mm_sched = meta["mm_sched"]
    grp_nmm = meta["grp_nmm"]
    NMM_TOT = meta["NMM_TOT"]

    nc = bacc.Bacc("TRN2", target_bir_lowering=False, debug=False,
                   num_devices=NCORES, num_swdge_queues=4)

    f32, bf16, i16 = mybir.dt.float32, mybir.dt.bfloat16, mybir.dt.int16
    AF = mybir.ActivationFunctionType
    OP = mybir.AluOpType

    xT_in = nc.dram_tensor("xT", [IN_DIM, SHARD_PAD], bf16, kind="ExternalInput")
    disq_in = nc.dram_tensor("disq", [P, NT], f32, kind="ExternalInput")
    idx_in = nc.dram_tensor("idx", [P, NIDX_TOT // 16], i16, kind="ExternalInput")
    sel_in = nc.dram_tensor("sel", [P, NMM_TOT * P], mybir.dt.float8e4, kind="ExternalInput")
    Wcat_in = nc.dram_tensor("Wcat", [IN_DIM, DCAT], bf16, kind="ExternalInput")
    bias_in = nc.dram_tensor("bias_rep", [P, DCAT], f32, kind="ExternalInput")
    wg_in = nc.dram_tensor("wg_rep", [P, DCAT], f32, kind="ExternalInput")
    bg_in = nc.dram_tensor("bg_rep", [P, 3], f32, kind="ExternalInput")
    out_ext = nc.dram_tensor("out", [SHARD_PAD, OUT_DIM], f32, kind="ExternalOutput")

    table_own_a = nc.dram_tensor("table_own_a", [HALF, D2], bf16)
    table_own_b = nc.dram_tensor("table_own_b", [SHARD_PAD - HALF, D2], bf16)
    wu_src = nc.dram_tensor("wu_src", [128, D2], bf16)
    wu_cin = nc.dram_tensor("wu_cin", [128, 8], bf16)
    wu_cout = nc.dram_tensor("wu_cout", [NCORES * 128, 8], bf16, addr_space="Shared")
    table_full_a = nc.dram_tensor("table_full_a", [NCORES * HALF, D2], bf16,
                                  addr_space="Shared")
    table_full_b = nc.dram_tensor("table_full_b", [NTOT_PAD - NCORES * HALF, D2],
                                  bf16, addr_space="Shared")

    with tile.TileContext(nc) as tc:
        with (
            tc.tile_pool(name="consts", bufs=1) as consts,
            tc.tile_pool(name="hpool", bufs=1) as hpool,
        ):
            # --- warmups: first collective and first SWDGE gather are cold
            # (ncfw staging / Q7 library load); run tiny dummies early so the
            # real ones hit warm paths, overlapped with phase 1 ---
            wu_idx = consts.tile([P, 8], mybir.dt.int16)
            nc.gpsimd.memset(wu_idx[:], 0)
            wu_out = consts.tile([P, 1, D2], bf16)
            nc.gpsimd.dma_gather(
                out_ap=wu_out[:], in_ap=wu_src[:], idxs_ap=wu_idx[:],
                num_idxs=128, num_idxs_reg=128, elem_size=D2,
                single_packet=False, queue_num=0)
            nc.gpsimd.collective_compute(
                "AllGather", mybir.AluOpType.bypass,
                replica_groups=[list(range(NCORES))],
                ins=[wu_cin[:]], outs=[wu_cout[:]],
            )
            W0 = consts.tile([P, DCAT], bf16)
            W1 = consts.tile([P, DCAT], bf16)
            nc.sync.dma_start(out=W0[:], in_=Wcat_in[0:P, :])
            nc.sync.dma_start(out=W1[:], in_=Wcat_in[P:2 * P, :])
            bias_t = consts.tile([P, DCAT], f32)
            nc.sync.dma_start(out=bias_t[:], in_=bias_in[:])
            wg_t = consts.tile([P, DCAT], f32)
            nc.sync.dma_start(out=wg_t[:], in_=wg_in[:])
            bg_t = consts.tile([P, 3], f32)
            nc.sync.dma_start(out=bg_t[:], in_=bg_in[:])
            disq_t = consts.tile([P, NT], f32)
            nc.sync.dma_start(out=disq_t[:], in_=disq_in[:])
            ndisq_t = consts.tile([P, NT], f32)
            nc.vector.tensor_scalar_mul(out=ndisq_t[:], in0=disq_t[:], scalar1=-1.0)

            # h_slab holds [h_hp | h_i] per tile: [128, 98*128] bf16
            h_slab = hpool.tile([P, NT * D2], bf16)

            # ---- phase 1 ----
            with (
                tc.tile_pool(name="xt", bufs=1) as xtp,
                tc.tile_pool(name="p1", bufs=3) as p1,
                tc.tile_pool(name="p1ps", bufs=2, space="PSUM") as p1ps,
            ):
                xt0 = xtp.tile([P, SHARD_PAD], bf16)
                xt1 = xtp.tile([P, SHARD_PAD], bf16)
                QW = SHARD_PAD // 4
                for qi in range(4):
                    nc.sync.dma_start(out=xt0[:, qi * QW:(qi + 1) * QW],
                                      in_=xT_in[0:P, qi * QW:(qi + 1) * QW])
                    nc.sync.dma_start(out=xt1[:, qi * QW:(qi + 1) * QW],
                                      in_=xT_in[P:2 * P, qi * QW:(qi + 1) * QW])
                for t in range(NT):
                    ps = p1ps.tile([P, DCAT], f32, tag="p1ps")
                    nc.tensor.matmul(out=ps[:], lhsT=xt0[:, t * P:(t + 1) * P],
                                     rhs=W0[:], start=True, stop=False)
                    nc.tensor.matmul(out=ps[:], lhsT=xt1[:, t * P:(t + 1) * P],
                                     rhs=W1[:], start=False, stop=True)
                    sc = p1.tile([P, DCAT], f32, tag="sc")
                    nc.vector.tensor_add(out=sc[:], in0=ps[:], in1=bias_t[:])
                    # h_slab tile t = [hp (0:64) | i (128:192)]
                    sc3 = sc[:].rearrange("p (a b) -> p a b", a=3)
                    nc.vector.tensor_copy(
                        out=h_slab[:, t * D2:(t + 1) * D2].rearrange(
                            "p (a b) -> p a b", a=2),
                        in_=sc3[:, 0::2, :])
                    tab = p1.tile([P, D2], bf16, tag="tab")
                    nc.scalar.activation(out=tab[:], in_=sc[:, 0:D2],
                                         func=AF.Copy, scale=disq_t[:, t:t + 1])
                    if t < HNT:
                        nc.sync.dma_start(out=table_own_a[t * P:(t + 1) * P, :],
                                          in_=tab[:])
                    else:
                        tb = t - HNT
                        nc.sync.dma_start(out=table_own_b[tb * P:(tb + 1) * P, :],
                                          in_=tab[:])
                    if t == HNT - 1:
                        nc.gpsimd.collective_compute(
                            "AllGather", mybir.AluOpType.bypass,
                            replica_groups=[list(range(NCORES))],
                            ins=[table_own_a[:]], outs=[table_full_a[:]],
                        )

            nc.gpsimd.collective_compute(
                "AllGather", mybir.AluOpType.bypass,
                replica_groups=[list(range(NCORES))],
                ins=[table_own_b[:]], outs=[table_full_b[:]],
            )

            # ---- phase 2 ----
            with (
                tc.tile_pool(name="gath", bufs=4) as gpool,
                tc.tile_pool(name="idxp", bufs=6) as idxp,
                tc.tile_pool(name="sel", bufs=2) as selp,
                tc.tile_pool(name="ep", bufs=3) as ep,
                tc.tile_pool(name="ps2", bufs=8, space="PSUM") as ps2,
            ):
                idx_off = 0
                mm_off = 0
                qn = 0
                max_nmm = int(grp_nmm.max())
                for g in range(NGRP):
                    ts = list(range(g * SUPER, min((g + 1) * SUPER, NT)))
                    nidx_g = int(grp_nidx[g].sum())
                    if nidx_g == 0:
                        continue
                    icols = nidx_g // 16
                    idx_t = idxp.tile([P, icols], i16, tag="idx")
                    nc.sync.dma_start(
                        out=idx_t[:],
                        in_=idx_in[:, idx_off // 16:idx_off // 16 + icols])
                    gts = []
                    off_in_g = 0
                    for b in range(NBUCK):
                        nb = int(grp_nidx[g, b])
                        if nb == 0:
                            gts.append(None)
                            continue
                        gt = gpool.tile([P, nb // P, D2], bf16, tag=f"g{b}")
                        if b < 2:
                            src_tab = table_full_a[b * BUCK:(b + 1) * BUCK, :]
                        else:
                            src_tab = table_full_b[(b - 2) * BUCK:(b - 1) * BUCK, :]
                        nc.gpsimd.dma_gather(
                            out_ap=gt[:],
                            in_ap=src_tab,
                            idxs_ap=idx_t[:, off_in_g // 16:(off_in_g + nb) // 16],
                            num_idxs=nb, num_idxs_reg=nb, elem_size=D2,
                            single_packet=False, queue_num=qn,
                        )
                        qn = (qn + 1) % 4
                        gts.append(gt)
                        off_in_g += nb
                    idx_off += nidx_g

                    nmm_g = int(grp_nmm[g])
                    selst = selp.tile([P, max_nmm, P], mybir.dt.float8e4, tag="selst")
                    sched_g = mm_sched[mm_off:mm_off + nmm_g]
                    nc.sync.dma_start(
                        out=selst[:, 0:nmm_g, :],
                        in_=sel_in[:, mm_off * P:(mm_off + nmm_g) * P])
                    n_touch = {t: 0 for t in ts}
                    for (gg, b, k, ti) in sched_g:
                        n_touch[ts[ti]] += 1
                    psums = {}
                    first = {t: True for t in ts}
                    done = {t: 0 for t in ts}
                    for m, (gg, b, k, ti) in enumerate(sched_g):
                        t = ts[ti]
                        if t not in psums:
                            psums[t] = ps2.tile([P, D2], f32, tag="acc",
                                                name=f"acc{t}")
                        done[t] += 1
                        nc.tensor.matmul(
                            out=psums[t][:], lhsT=selst[:, m, :],
                            rhs=gts[b][:, k, :],
                            start=first[t],
                            stop=(done[t] == n_touch[t]))
                        first[t] = False
                    mm_off += nmm_g

                    for t in ts:
                        acc = psums[t]
                        hof = t * D2
                        Hcat = ep.tile([P, DCAT], f32, tag="Hcat")
                        nc.vector.scalar_tensor_tensor(
                            out=Hcat[:, 0:OUT_DIM], in0=acc[:, 0:OUT_DIM],
                            scalar=ndisq_t[:, t:t + 1],
                            in1=h_slab[:, hof:hof + OUT_DIM],
                            op0=OP.mult, op1=OP.add)
                        nc.scalar.activation(out=Hcat[:, 0:OUT_DIM],
                                             in_=Hcat[:, 0:OUT_DIM], func=AF.Relu)
                        nc.scalar.activation(out=Hcat[:, OUT_DIM:D2],
                                             in_=acc[:, OUT_DIM:D2], func=AF.Relu,
                                             scale=disq_t[:, t:t + 1])
                        nc.scalar.activation(out=Hcat[:, D2:DCAT],
                                             in_=h_slab[:, hof + OUT_DIM:hof + D2],
                                             func=AF.Relu)
                        gm = ep.tile([P, DCAT], f32, tag="gm")
                        nc.vector.tensor_tensor(out=gm[:], in0=Hcat[:], in1=wg_t[:],
                                                op=OP.mult)
                        g3 = ep.tile([P, 4], f32, tag="g3")
                        nc.vector.reduce_sum(
                            out=g3[:, 0:3],
                            in_=gm[:].rearrange("p (a b) -> p a b", a=3),
                            axis=mybir.AxisListType.X)
                        nc.vector.tensor_add(out=g3[:, 0:3], in0=g3[:, 0:3],
                                             in1=bg_t[:])
                        o = ep.tile([P, OUT_DIM], f32, tag="o")
                        nc.scalar.activation(out=o[:], in_=Hcat[:, 0:OUT_DIM],
                                             func=AF.Copy, scale=g3[:, 0:1])
                        nc.vector.scalar_tensor_tensor(
                            out=o[:], in0=Hcat[:, OUT_DIM:D2], scalar=g3[:, 1:2],
                            in1=o[:], op0=OP.mult, op1=OP.add)
                        nc.vector.scalar_tensor_tensor(
                            out=o[:], in0=Hcat[:, D2:DCAT], scalar=g3[:, 2:3],
                            in1=o[:], op0=OP.mult, op1=OP.add)
                        mx = ep.tile([P, 4], f32, tag="mx")
                        nc.vector.tensor_reduce(out=mx[:, 0:1], in_=o[:],
                                                op=OP.max,
                                                axis=mybir.AxisListType.X,
                                                negate=True)
                        et = ep.tile([P, OUT_DIM], f32, tag="et")
                        nc.scalar.activation(out=et[:], in_=o[:], func=AF.Exp,
                                             bias=mx[:, 0:1], scale=1.0,
                                             accum_out=mx[:, 1:2])
                        nc.scalar.activation(out=mx[:, 2:3], in_=mx[:, 1:2],
                                             func=AF.Ln)
                        fin = ep.tile([P, OUT_DIM], f32, tag="fin")
                        nc.vector.tensor_scalar(
                            out=fin[:], in0=o[:], scalar1=mx[:, 0:1],
                            scalar2=mx[:, 2:3], op0=OP.add, op1=OP.subtract)
                        nc.sync.dma_start(out=out_ext[t * P:(t + 1) * P, :],
                                          in_=fin[:])

    nc.compile()
    return nc


def kernel(**inputs) -> np.ndarray:
    consts, meta, per_core = _build_host_data(**inputs)
    nc = _build_bass(meta)

    in_maps = []
    for c in range(NCORES):
        in_maps.append(dict(
            xT=np.ascontiguousarray(per_core["xT"][c]),
            disq=np.ascontiguousarray(per_core["disq_col"][c]),
            idx=np.ascontiguousarray(per_core["idx_wrapped"][c]),
            sel=per_core["sel_stream"][c].reshape(P, -1),
            Wcat=consts["Wcat"], bias_rep=consts["bias_rep"],
            wg_rep=consts["wg_rep"], bg_rep=consts["bg_rep"],
        ))

    from concourse.bass_utils import run_bass_kernel_spmd
    res = run_bass_kernel_spmd(nc, in_maps, core_ids=list(range(NCORES)))
    out = np.concatenate([res.results[c]["out"][:SHARD] for c in range(NCORES)],
                         axis=0)
    return out.astype(np.float32)


if __name__ == "__main__":
    import reference
    ins = {k: np.asarray(v) for k, v in reference.setup_inputs().items()}
    got = kernel(**ins)
    exp = np.asarray(reference.reference(**reference.setup_inputs()))
    rel = np.linalg.norm(got - exp) / np.linalg.norm(exp)
    print("Relative error:", rel)


# revision 34
# speedup vs baseline: 1.3278x; 1.0494x over previous
"""ACM-GCN single-layer kernel for Trainium2, 8 NeuronCores (SPMD).

Strategy (graph/data parallel):
- Nodes partitioned 12500/core (padded to 12544 = 98*128).
- Phase 1: h = x_shard @ [W_hp|W_lp|W_i] + b (bf16 PE matmul); h_hp/h_i kept
  in SBUF; deg_isqrt-prescaled bf16 table [12544, 128] written to DRAM.
- AllGather the table -> full [100352, 128] bf16 table per core.
- Phase 2: per dst tile, dma_gather the source rows of its edges (4 SWDGE
  queues, int16 indices via 4 source buckets), build one-hot selection
  matrices (tensor_scalar is_equal vs iota -> DVE 4x mode) and accumulate
  sel.T @ gathered in PSUM.  Self-edges appended so the self-loop term is
  folded into the aggregation.  Epilogue fuses HP/LP/I branches, gates and
  log_softmax.
"""
import numpy as np
import ml_dtypes

N_NODES = 100000
N_EDGES = 3200000
IN_DIM = 256
OUT_DIM = 64
NCORES = 8
P = 128
SHARD = N_NODES // NCORES            # 12500
NT = (SHARD + P - 1) // P            # 98 tiles
SHARD_PAD = NT * P                   # 12544
NTOT_PAD = SHARD_PAD * NCORES        # 100352
NBUCK = 4
BUCK = NTOT_PAD // NBUCK             # 25088 (< 32768, int16-safe)
D2 = 2 * OUT_DIM                     # 128 gathered feature dim (hp|lp)
DCAT = 3 * OUT_DIM                   # 192
SUPER = 2                            # dst tiles per gather group
HNT = NT // 2                        # 49 tiles per half
HALF = HNT * P                       # 6272 rows per half
BF16 = ml_dtypes.bfloat16


def _build_host_data(x, edge_index, W_hp, b_hp, W_lp, b_lp, W_i, b_i,
                     w_gh, b_gh, w_gl, b_gl, w_gi, b_gi):
    src = np.asarray(edge_index[0], dtype=np.int64)
    dst = np.asarray(edge_index[1], dtype=np.int64)

    deg = np.bincount(dst, minlength=N_NODES).astype(np.float64) + 1.0
    disqrt = (1.0 / np.sqrt(deg)).astype(np.float32)

    # self edges: with the prescaled table they contribute deg_inv * h
    allv = np.arange(N_NODES, dtype=np.int64)
    src = np.concatenate([src, allv])
    dst = np.concatenate([dst, allv])

    core = dst // SHARD
    dloc = dst - core * SHARD
    tile = dloc // P
    dstloc = (dloc % P).astype(np.int32)
    # half-major padded global index: the table is all-gathered as two
    # per-half collectives, so global row = half*8*HALF + core*HALF + loc%HALF
    s_core = src // SHARD
    s_loc = src % SHARD
    s_half = s_loc // HALF
    spad = s_half * (NCORES * HALF) + s_core * HALF + (s_loc - s_half * HALF)
    buck = (spad // BUCK).astype(np.int32)
    idx16 = (spad - buck.astype(np.int64) * BUCK).astype(np.int16)

    key = ((core * NT + tile) * NBUCK + buck).astype(np.int64)
    order = np.argsort(key, kind="stable")
    idx16_s = idx16[order]
    dstloc_s = dstloc[order]
    cnts = np.bincount(key[order], minlength=NCORES * NT * NBUCK).reshape(
        NCORES, NT, NBUCK)

    NGRP = (NT + SUPER - 1) // SUPER
    # pack each (group, bucket)'s edges contiguously (tiles concatenated, one
    # pad-to-128 at the end); dl encodes tile-within-group*128 + dstloc
    grp_cnt = np.zeros((NCORES, NGRP, NBUCK), np.int64)
    for g in range(NGRP):
        ts = list(range(g * SUPER, min((g + 1) * SUPER, NT)))
        grp_cnt[:, g, :] = cnts[:, ts, :].sum(axis=1)
    C_gb = np.ceil(grp_cnt.max(axis=0) / P).astype(np.int64)       # [NGRP, NBUCK]
    NCHUNK_TOT = int(C_gb.sum())
    NIDX_TOT = NCHUNK_TOT * P
    grp_nidx = C_gb * P

    core_seg_start = np.cumsum(cnts.reshape(NCORES, -1), axis=1).reshape(
        NCORES, NT, NBUCK) - cnts
    core_base = np.concatenate([[0], np.cumsum(cnts.sum(axis=(1, 2)))])[:-1]

    idx_stream = np.zeros((NCORES, NIDX_TOT), np.int16)
    dl_stream = np.full((NCORES, NIDX_TOT), 300.0, np.float32)
    pos = 0
    for g in range(NGRP):
        ts = list(range(g * SUPER, min((g + 1) * SUPER, NT)))
        for b in range(NBUCK):
            n_pad = int(C_gb[g, b]) * P
            if n_pad == 0:
                continue
            for c in range(NCORES):
                p0 = pos
                for ti, t in enumerate(ts):
                    s0 = core_base[c] + core_seg_start[c, t, b]
                    n = int(cnts[c, t, b])
                    idx_stream[c, p0:p0 + n] = idx16_s[s0:s0 + n]
                    dl_stream[c, p0:p0 + n] = dstloc_s[s0:s0 + n] + ti * P
                    p0 += n
            pos += n_pad
    assert pos == NIDX_TOT

    idx_wrapped = np.zeros((NCORES, 128, NIDX_TOT // 16), np.int16)
    for c in range(NCORES):
        a = idx_stream[c].reshape(NIDX_TOT // 16, 16).T
        idx_wrapped[c] = np.tile(a, (8, 1))

    # matmul schedule: per (g, b, chunk) the set of group-local tiles it can
    # touch on any core; one streamed one-hot sel block per (chunk, tile) pair
    mm_sched = []          # (g, b, k, ti)
    grp_nmm = np.zeros(NGRP, np.int64)
    for g in range(NGRP):
        ts = list(range(g * SUPER, min((g + 1) * SUPER, NT)))
        nmm = 0
        for b in range(NBUCK):
            run_min = np.zeros(len(ts) + 1, np.int64)
            run_max = np.zeros(len(ts) + 1, np.int64)
            for ti, t in enumerate(ts):
                run_min[ti + 1] = run_min[ti] + cnts[:, t, b].min()
                run_max[ti + 1] = run_max[ti] + cnts[:, t, b].max()
            for k in range(int(C_gb[g, b])):
                ks, ke = k * P, (k + 1) * P
                for ti, t in enumerate(ts):
                    if ke > run_min[ti] and ks < run_max[ti + 1]:
                        mm_sched.append((g, b, k, ti))
                        nmm += 1
        grp_nmm[g] = nmm
    NMM_TOT = len(mm_sched)

    # chunk start offsets in the idx/dl stream, per (g, b)
    gb_off = {}
    off = 0
    for g in range(NGRP):
        for b in range(NBUCK):
            gb_off[(g, b)] = off
            off += int(C_gb[g, b]) * P

    ONE = np.float32(1.0).astype(ml_dtypes.float8_e4m3).view(np.uint8)
    sel_stream = np.zeros((NCORES, 128, NMM_TOT * P), np.uint8)
    dvals = np.arange(P, dtype=np.float32)
    for m, (g, b, k, ti) in enumerate(mm_sched):
        base = gb_off[(g, b)] + k * P
        for c in range(NCORES):
            col = dl_stream[c, base:base + P] - ti * P       # [128 edges]
            mask = (col >= 0) & (col < P)
            pp = np.nonzero(mask)[0]
            sel_stream[c, pp, m * P + col[pp].astype(np.int64)] = ONE
    sel_stream = sel_stream.view(ml_dtypes.float8_e4m3)

    W_cat = np.concatenate([W_hp, W_lp, W_i], axis=1).astype(np.float32)
    b_cat = np.concatenate([b_hp, b_lp, b_i]).astype(np.float32)
    wg_cat = np.concatenate([w_gh[:, 0], w_gl[:, 0], w_gi[:, 0]]).astype(np.float32)
    bg_cat = np.array([b_gh[0], b_gl[0], b_gi[0]], np.float32)

    xT = np.zeros((NCORES, IN_DIM, SHARD_PAD), BF16)
    disq_col = np.ones((NCORES, P, NT), np.float32)
    x = np.asarray(x, np.float32)
    for c in range(NCORES):
        xT[c, :, :SHARD] = x[c * SHARD:(c + 1) * SHARD].T.astype(BF16)
        d = np.ones(SHARD_PAD, np.float32)
        d[:SHARD] = disqrt[c * SHARD:(c + 1) * SHARD]
        disq_col[c] = d.reshape(NT, P).T

    consts = dict(
        Wcat=W_cat.astype(BF16),
        bias_rep=np.tile(b_cat[None, :], (P, 1)).astype(np.float32),
        wg_rep=np.tile(wg_cat[None, :], (P, 1)).astype(np.float32),
        bg_rep=np.tile(bg_cat[None, :], (P, 1)).astype(np.float32),
        iota=np.tile(np.arange(SUPER * P, dtype=np.float32)[None, :],
                     (P, 1)).astype(BF16),
    )
    meta = dict(C_gb=C_gb, grp_cnt=grp_cnt, cnts=cnts, grp_nidx=grp_nidx,
                NCHUNK_TOT=NCHUNK_TOT, NIDX_TOT=NIDX_TOT, NGRP=NGRP,
                mm_sched=mm_sched, grp_nmm=grp_nmm, NMM_TOT=NMM_TOT)
    per_core = dict(xT=xT, disq_col=disq_col, idx_wrapped=idx_wrapped,
                    sel_stream=sel_stream)
    return consts, meta, per_core


def _force_act_set():
    """Make every activation use the one table set that holds relu+exp+ln+copy
    (index preserved), so the kernel loads the ACT table exactly once."""
    import concourse.hw_specs as hw_specs
    if getattr(hw_specs, "_acm_patched", False):
        return
    orig = hw_specs.get_activation_tables

    def patched(module_arch):
        tabs = orig(module_arch)
        full = None
        for name, funcs in tabs.items():
            fn = {str(f) for f in funcs}
            if any("Exp" in f for f in fn) and any("Ln" in str(f) for f in fn) \
               and any("Relu" in f for f in fn):
                full = name
                break
        if full is None:
            return tabs
        keep = tabs[full]
        return {name: (funcs if name == full else (funcs & keep) - keep)
                for name, funcs in tabs.items()}

    hw_specs.get_activation_tables = patched
    import concourse.bacc as bacc_mod
    bacc_mod.get_activation_tables = patched
    hw_specs._acm_patched = True


def _build_bass(meta):
    import concourse.bacc as bacc
    import concourse.tile as tile
    from concourse import mybir

    _force_act_set()

    C_gb = meta["C_gb"]
    grp_cnt = meta["grp_cnt"]
    cnts = meta["cnts"]
    grp_nidx = meta["grp_nidx"]
    NCHUNK_TOT = meta["NCHUNK_TOT"]
    NIDX_TOT = meta["NIDX_TOT"]
    NGRP = meta["NGRP"]
    mm_sched = meta["mm_sched"]
    grp_nmm = meta["grp_nmm"]
    NMM_TOT = meta["NMM_TOT"]

    nc = bacc.Bacc("TRN2", target_bir_lowering=False, debug=False,
                   num_devices=NCORES, num_swdge_queues=4)

    f32, bf16, i16 = mybir.dt.float32, mybir.dt.bfloat16, mybir.dt.int16
    AF = mybir.ActivationFunctionType
    OP = mybir.AluOpType

    xT_in = nc.dram_tensor("xT", [IN_DIM, SHARD_PAD], bf16, kind="ExternalInput")
    disq_in = nc.dram_tensor("disq", [P, NT], f32, kind="ExternalInput")
    idx_in = nc.dram_tensor("idx", [P, NIDX_TOT // 16], i16, kind="ExternalInput")
    sel_in = nc.dram_tensor("sel", [P, NMM_TOT * P], mybir.dt.float8e4, kind="ExternalInput")
    Wcat_in = nc.dram_tensor("Wcat", [IN_DIM, DCAT], bf16, kind="ExternalInput")
    bias_in = nc.dram_tensor("bias_rep", [P, DCAT], f32, kind="ExternalInput")
    wg_in = nc.dram_tensor("wg_rep", [P, DCAT], f32, kind="ExternalInput")
    bg_in = nc.dram_tensor("bg_rep", [P, 3], f32, kind="ExternalInput")
    out_ext = nc.dram_tensor("out", [SHARD_PAD, OUT_DIM], f32, kind="ExternalOutput")

    table_own_a = nc.dram_tensor("table_own_a", [HALF, D2], bf16)
    table_own_b = nc.dram_tensor("table_own_b", [SHARD_PAD - HALF, D2], bf16)
    wu_src = nc.dram_tensor("wu_src", [128, D2], bf16)
    wu_cin = nc.dram_tensor("wu_cin", [128, 8], bf16)
    wu_cout = nc.dram_tensor("wu_cout", [NCORES * 128, 8], bf16, addr_space="Shared")
    table_full_a = nc.dram_tensor("table_full_a", [NCORES * HALF, D2], bf16,
                                  addr_space="Shared")
    table_full_b = nc.dram_tensor("table_full_b", [NTOT_PAD - NCORES * HALF, D2],
                                  bf16, addr_space="Shared")

    with tile.TileContext(nc) as tc:
        with (
            tc.tile_pool(name="consts", bufs=1) as consts,
            tc.tile_pool(name="hpool", bufs=1) as hpool,
        ):
            # --- warmups: first collective and first SWDGE gather are cold
            # (ncfw staging / Q7 library load); run tiny dummies early so the
            # real ones hit warm paths, overlapped with phase 1 ---
            wu_idx = consts.tile([P, 8], mybir.dt.int16)
            nc.gpsimd.memset(wu_idx[:], 0)
            wu_out = consts.tile([P, 1, D2], bf16)
            nc.gpsimd.dma_gather(
                out_ap=wu_out[:], in_ap=wu_src[:], idxs_ap=wu_idx[:],
                num_idxs=128, num_idxs_reg=128, elem_size=D2,
                single_packet=False, queue_num=0)
            nc.gpsimd.collective_compute(
                "AllGather", mybir.AluOpType.bypass,
                replica_groups=[list(range(NCORES))],
                ins=[wu_cin[:]], outs=[wu_cout[:]],
            )
            W0 = consts.tile([P, DCAT], bf16)
            W1 = consts.tile([P, DCAT], bf16)
            nc.sync.dma_start(out=W0[:], in_=Wcat_in[0:P, :])
            nc.sync.dma_start(out=W1[:], in_=Wcat_in[P:2 * P, :])
            bias_t = consts.tile([P, DCAT], f32)
            nc.sync.dma_start(out=bias_t[:], in_=bias_in[:])
            wg_t = consts.tile([P, DCAT], f32)
            nc.sync.dma_start(out=wg_t[:], in_=wg_in[:])
            bg_t = consts.tile([P, 3], f32)
            nc.sync.dma_start(out=bg_t[:], in_=bg_in[:])
            disq_t = consts.tile([P, NT], f32)
            nc.sync.dma_start(out=disq_t[:], in_=disq_in[:])
            ndisq_t = consts.tile([P, NT], f32)
            nc.vector.tensor_scalar_mul(out=ndisq_t[:], in0=disq_t[:], scalar1=-1.0)

            # h_slab holds [h_hp | h_i] per tile: [128, 98*128] bf16
            h_slab = hpool.tile([P, NT * D2], bf16)

            # ---- phase 1 ----
            with (
                tc.tile_pool(name="xt", bufs=1) as xtp,
                tc.tile_pool(name="p1", bufs=3) as p1,
                tc.tile_pool(name="p1ps", bufs=2, space="PSUM") as p1ps,
            ):
                xt0 = xtp.tile([P, SHARD_PAD], bf16)
                xt1 = xtp.tile([P, SHARD_PAD], bf16)
                QW = SHARD_PAD // 4
                for qi in range(4):
                    nc.sync.dma_start(out=xt0[:, qi * QW:(qi + 1) * QW],
                                      in_=xT_in[0:P, qi * QW:(qi + 1) * QW])
                    nc.sync.dma_start(out=xt1[:, qi * QW:(qi + 1) * QW],
                                      in_=xT_in[P:2 * P, qi * QW:(qi + 1) * QW])
                for t in range(NT):
                    ps = p1ps.tile([P, DCAT], f32, tag="p1ps")
                    nc.tensor.matmul(out=ps[:], lhsT=xt0[:, t * P:(t + 1) * P],
                                     rhs=W0[:], start=True, stop=False)
                    nc.tensor.matmul(out=ps[:], lhsT=xt1[:, t * P:(t + 1) * P],
                                     rhs=W1[:], start=False, stop=True)
                    sc = p1.tile([P, DCAT], f32, tag="sc")
                    nc.vector.tensor_add(out=sc[:], in0=ps[:], in1=bias_t[:])
                    # h_slab tile t = [hp (0:64) | i (128:192)]
                    sc3 = sc[:].rearrange("p (a b) -> p a b", a=3)
                    nc.vector.tensor_copy(
                        out=h_slab[:, t * D2:(t + 1) * D2].rearrange(
                            "p (a b) -> p a b", a=2),
                        in_=sc3[:, 0::2, :])
                    tab = p1.tile([P, D2], bf16, tag="tab")
                    nc.scalar.activation(out=tab[:], in_=sc[:, 0:D2],
                                         func=AF.Copy, scale=disq_t[:, t:t + 1])
                    if t < HNT:
                        nc.sync.dma_start(out=table_own_a[t * P:(t + 1) * P, :],
                                          in_=tab[:])
                    else:
                        tb = t - HNT
                        nc.sync.dma_start(out=table_own_b[tb * P:(tb + 1) * P, :],
                                          in_=tab[:])
                    if t == HNT - 1:
                        nc.gpsimd.collective_compute(
                            "AllGather", mybir.AluOpType.bypass,
                            replica_groups=[list(range(NCORES))],
                            ins=[table_own_a[:]], outs=[table_full_a[:]],
                        )

            nc.gpsimd.collective_compute(
                "AllGather", mybir.AluOpType.bypass,
                replica_groups=[list(range(NCORES))],
                ins=[table_own_b[:]], outs=[table_full_b[:]],
            )

            # ---- phase 2 ----
            with (
                tc.tile_pool(name="gath", bufs=4) as gpool,
                tc.tile_pool(name="idxp", bufs=6) as idxp,
                tc.tile_pool(name="sel", bufs=3) as selp,
                tc.tile_pool(name="ep", bufs=3) as ep,
                tc.tile_pool(name="ps2", bufs=8, space="PSUM") as ps2,
            ):
                idx_off = 0
                mm_off = 0
                qn = 0
                max_nmm = int(grp_nmm.max())
                for g in range(NGRP):
                    ts = list(range(g * SUPER, min((g + 1) * SUPER, NT)))
                    nidx_g = int(grp_nidx[g].sum())
                    if nidx_g == 0:
                        continue
                    icols = nidx_g // 16
                    idx_t = idxp.tile([P, icols], i16, tag="idx")
                    nc.sync.dma_start(
                        out=idx_t[:],
                        in_=idx_in[:, idx_off // 16:idx_off // 16 + icols])
                    gts = []
                    off_in_g = 0
                    for b in range(NBUCK):
                        nb = int(grp_nidx[g, b])
                        if nb == 0:
                            gts.append(None)
                            continue
                        gt = gpool.tile([P, nb // P, D2], bf16, tag=f"g{b}")
                        if b < 2:
                            src_tab = table_full_a[b * BUCK:(b + 1) * BUCK, :]
                        else:
                            src_tab = table_full_b[(b - 2) * BUCK:(b - 1) * BUCK, :]
                        nc.gpsimd.dma_gather(
                            out_ap=gt[:],
                            in_ap=src_tab,
                            idxs_ap=idx_t[:, off_in_g // 16:(off_in_g + nb) // 16],
                            num_idxs=nb, num_idxs_reg=nb, elem_size=D2,
                            single_packet=False, queue_num=qn,
                        )
                        qn = (qn + 1) % 4
                        gts.append(gt)
                        off_in_g += nb
                    idx_off += nidx_g

                    nmm_g = int(grp_nmm[g])
                    selst = selp.tile([P, max_nmm, P], mybir.dt.float8e4, tag="selst")
                    sched_g = mm_sched[mm_off:mm_off + nmm_g]
                    nc.sync.dma_start(
                        out=selst[:, 0:nmm_g, :],
                        in_=sel_in[:, mm_off * P:(mm_off + nmm_g) * P])
                    n_touch = {t: 0 for t in ts}
                    for (gg, b, k, ti) in sched_g:
                        n_touch[ts[ti]] += 1
                    psums = {}
                    first = {t: True for t in ts}
                    done = {t: 0 for t in ts}
                    for m, (gg, b, k, ti) in enumerate(sched_g):
                        t = ts[ti]
                        if t not in psums:
                            psums[t] = ps2.tile([P, D2], f32, tag="acc",
                                                name=f"acc{t}")
                        done[t] += 1
                        nc.tensor.matmul(
                            out=psums[t][:], lhsT=selst[:, m, :],
                            rhs=gts[b][:, k, :],
                            start=first[t],
                            stop=(done[t] == n_touch[t]))
                        first[t] = False
                    mm_off += nmm_g

                    for t in ts:
                        acc = psums[t]
                        hof = t * D2
                        Hcat = ep.tile([P, DCAT], f32, tag="Hcat")
                        nc.vector.scalar_tensor_tensor(
                            out=Hcat[:, 0:OUT_DIM], in0=acc[:, 0:OUT_DIM],
                            scalar=ndisq_t[:, t:t + 1],
                            in1=h_slab[:, hof:hof + OUT_DIM],
                            op0=OP.mult, op1=OP.add)
                        nc.scalar.activation(out=Hcat[:, 0:OUT_DIM],
                                             in_=Hcat[:, 0:OUT_DIM], func=AF.Relu)
                        nc.scalar.activation(out=Hcat[:, OUT_DIM:D2],
                                             in_=acc[:, OUT_DIM:D2], func=AF.Relu,
                                             scale=disq_t[:, t:t + 1])
                        nc.scalar.activation(out=Hcat[:, D2:DCAT],
                                             in_=h_slab[:, hof + OUT_DIM:hof + D2],
                                             func=AF.Relu)
                        gm = ep.tile([P, DCAT], f32, tag="gm")
                        nc.vector.tensor_tensor(out=gm[:], in0=Hcat[:], in1=wg_t[:],
                                                op=OP.mult)
                        g3 = ep.tile([P, 4], f32, tag="g3")
                        nc.vector.reduce_sum(
                            out=g3[:, 0:3],
                            in_=gm[:].rearrange("p (a b) -> p a b", a=3),
                            axis=mybir.AxisListType.X)
                        nc.vector.tensor_add(out=g3[:, 0:3], in0=g3[:, 0:3],
                                             in1=bg_t[:])
                        o = ep.tile([P, OUT_DIM], f32, tag="o")
                        nc.scalar.activation(out=o[:], in_=Hcat[:, 0:OUT_DIM],
                                             func=AF.Copy, scale=g3[:, 0:1])
                        nc.vector.scalar_tensor_tensor(
                            out=o[:], in0=Hcat[:, OUT_DIM:D2], scalar=g3[:, 1:2],
                            in1=o[:], op0=OP.mult, op1=OP.add)
                        nc.vector.scalar_tensor_tensor(
                            out=o[:], in0=Hcat[:, D2:DCAT], scalar=g3[:, 2:3],
                            in1=o[:], op0=OP.mult, op1=OP.add)
                        mx = ep.tile([P, 4], f32, tag="mx")
                        nc.vector.tensor_reduce(out=mx[:, 0:1], in_=o[:],
                                                op=OP.max,
                                                axis=mybir.AxisListType.X,
                                                negate=True)
                        et = ep.tile([P, OUT_DIM], f32, tag="et")
                        nc.scalar.activation(out=et[:], in_=o[:], func=AF.Exp,
                                             bias=mx[:, 0:1], scale=1.0,
                                             accum_out=mx[:, 1:2])
                        nc.scalar.activation(out=mx[:, 2:3], in_=mx[:, 1:2],
                                             func=AF.Ln)
                        fin = ep.tile([P, OUT_DIM], f32, tag="fin")
                        nc.vector.tensor_scalar(
                            out=fin[:], in0=o[:], scalar1=mx[:, 0:1],
                            scalar2=mx[:, 2:3], op0=OP.add, op1=OP.subtract)
                        nc.sync.dma_start(out=out_ext[t * P:(t + 1) * P, :],
                                          in_=fin[:])

    nc.compile()
    return nc


def kernel(**inputs) -> np.ndarray:
    consts, meta, per_core = _build_host_data(**inputs)
    nc = _build_bass(meta)

    in_maps = []
    for c in range(NCORES):
        in_maps.append(dict(
            xT=np.ascontiguousarray(per_core["xT"][c]),
            disq=np.ascontiguousarray(per_core["disq_col"][c]),
            idx=np.ascontiguousarray(per_core["idx_wrapped"][c]),
            sel=per_core["sel_stream"][c].reshape(P, -1),
            Wcat=consts["Wcat"], bias_rep=consts["bias_rep"],
            wg_rep=consts["wg_rep"], bg_rep=consts["bg_rep"],
        ))

    from concourse.bass_utils import run_bass_kernel_spmd
    res = run_bass_kernel_spmd(nc, in_maps, core_ids=list(range(NCORES)))
    out = np.concatenate([res.results[c]["out"][:SHARD] for c in range(NCORES)],
                         axis=0)
    return out.astype(np.float32)


if __name__ == "__main__":
    import reference
    ins = {k: np.asarray(v) for k, v in reference.setup_inputs().items()}
    got = kernel(**ins)
    exp = np.asarray(reference.reference(**reference.setup_inputs()))
    rel = np.linalg.norm(got - exp) / np.linalg.norm(exp)
    print("Relative error:", rel)
